# revision 63
# baseline (speedup 1.0000x reference)
"""Multi-head attention (B=2, S=2048, D=1024, H=16, causal) on 8 TRN2 cores,
head-parallel: each core computes 2 heads' q/k/v + attention and a partial
output projection; host sums the 8 partials and adds bo.

v2 (bf16): all matmul operands are bf16 (cost-model rate 1 cycle/row at any
moving size; rel-err budget 2e-2 >> bf16 error). HBM traffic is halved and
packed into a handful of large contiguous DMAs via host-side layout:

  xp   (128, 8*8*512)  x^T packed chunk-major: col = t*4096 + dc*512 + c
  wqkv (128, 8*384)    per-dc blocks [wq/sqrt(dh) | wk | wv] columns
  out  (128, 32*1024)  row-block-major: col = blk*1024 + d, blk = token//128

V is projected in flipped orientation (stationary = x sub-chunk, moving = wv
columns) so it lands directly in the [key, dim] layout attention needs - no
PE transposes. Its bias is folded in as a K=1 ones-row matmul. Scores are
computed transposed (S^T[k, q]) per head into halves of one [128,1024] psum
tile so exp / causal-select run once per k-block pair. Normalization: 1/denom
rows (from a ones-column in vaug) broadcast via a tiny ones-matmul, one fused
multiply per chunk; the k-block order puts a full-width block first (psum
start covers every column) and full-width last where possible (clean stop).
"""

import numpy as np
import ml_dtypes

import concourse.bass as bass
import concourse.tile as tile
from concourse import bacc, mybir
from concourse.bass_utils import run_bass_kernel_spmd

B, S, D, H = 2, 2048, 1024, 16
DH = D // H  # 64
NCORES = 8
HPC = H // NCORES  # 2 heads per core
T = B * S  # 4096
QCH = 512
KCH = 128
NQC = S // QCH  # 4
NKC = S // KCH  # 16
NTC = T // QCH  # 8
ND = D // 128  # 8
XC = ND * QCH  # 4096 packed-x columns per token chunk
VW = 65  # vaug block width (64 dims + ones column)

f32 = mybir.dt.float32
bf16 = mybir.dt.bfloat16
AF = mybir.ActivationFunctionType
ALU = mybir.AluOpType
BF = ml_dtypes.bfloat16

PIPE = 12  # score->exp->PV pipeline depth in k-blocks
PEND_AT = 7  # loop index from which deferred phase-C pieces are drained
FILL_POPS = 1  # phase-A units drained per k-block
LOOKAHEAD_EXTRA = 0  # extra chunks of phase-A queued beyond req+1
PT_BUFS = 14  # exp-output tiles in flight


def _classify_blocks(mask):
    """mask: (S, S) bool [q, k] -> dict (qc, kc) -> (kind, mixed_idx)."""
    blocks = {}
    qg, kg = np.meshgrid(np.arange(S), np.arange(S), indexing="ij")
    causal = qg >= kg
    n_mixed = 0
    for qc in range(NQC):
        for kc in range(NKC):
            reg = mask[qc * QCH : (qc + 1) * QCH, kc * KCH : (kc + 1) * KCH]
            if not reg.any():
                blocks[(qc, kc)] = ("none", -1)
            elif reg.all():
                blocks[(qc, kc)] = ("all", -1)
            elif np.array_equal(
                reg, causal[qc * QCH : (qc + 1) * QCH, kc * KCH : (kc + 1) * KCH]
            ):
                blocks[(qc, kc)] = ("causal", -1)
            else:
                blocks[(qc, kc)] = ("mixed", n_mixed)
                n_mixed += 1
    return blocks, n_mixed


def _order_kcs(blocks, qc):
    """k-block emission order: a full-width block first (its psum write starts
    every column), full-width blocks in the middle, and when possible a
    full-width block last (clean accumulation-group stop)."""
    kcs = [kc for kc in range(NKC) if blocks[(qc, kc)][0] != "none"]
    if not kcs:
        return []

    def f0_of(kc):
        kind, _ = blocks[(qc, kc)]
        return max(0, kc * KCH - qc * QCH) if kind == "causal" else 0

    full = [kc for kc in kcs if f0_of(kc) == 0]
    trimmed = sorted((kc for kc in kcs if f0_of(kc) > 0), key=f0_of, reverse=True)
    assert full, f"q-chunk {qc} has no full-width block"
    if len(full) == 1:
        return [full[0]] + trimmed  # sloppy stop (skip_group_check)
    return full[:-1] + trimmed + [full[-1]]


def _build(mask, reps=1):
    blocks, n_mixed = _classify_blocks(mask)

    nc = bacc.Bacc("TRN2", target_bir_lowering=False, debug=False, num_devices=NCORES)
    x_d = nc.dram_tensor("xp", (128, NTC * XC), bf16, kind="ExternalInput").ap()
    w_d = nc.dram_tensor("wqkv", (128, ND * 384), bf16, kind="ExternalInput").ap()
    bqk_d = nc.dram_tensor("bqk", (128, 2), f32, kind="ExternalInput").ap()
    bvo_d = nc.dram_tensor("bvo", (1, 512), bf16, kind="ExternalInput").ap()
    wo_d = nc.dram_tensor("wot", (128, D), bf16, kind="ExternalInput").ap()
    out_d = nc.dram_tensor("out", (128, (T // 128) * D), bf16, kind="ExternalOutput").ap()
    if n_mixed:
        mb_d = nc.dram_tensor("mblk", (n_mixed * 128, QCH), bf16, kind="ExternalInput").ap()

    def pair_ap(t, f0, width):
        """[128, (2 heads, width)] view of a [128, 1024] tile at column f0."""
        return bass.AP(t.tensor, t.offset + f0, [t.ap[0], [512, 2], [1, width]])

    with tile.TileContext(nc) as tc:
        with (
            tc.tile_pool(name="const", bufs=1) as cpool,
            tc.tile_pool(name="work", bufs=1) as wpool,
            tc.tile_pool(name="psum", bufs=1, space="PSUM") as ppool,
        ):
            # ---- input stream: interleave weights and early x so the first
            # projection matmuls start as soon as (w-dc0/1, x-chunk0) land ----
            w = cpool.tile([128, ND * 384], bf16, name="w")
            xall = cpool.tile([128, NTC * XC], bf16, name="xall")
            nc.sync.dma_start(w[:, 0:768], w_d[:, 0:768])
            nc.sync.dma_start(xall[:, 0:1024], x_d[:, 0:1024])
            nc.sync.dma_start(w[:, 768:1536], w_d[:, 768:1536])
            nc.sync.dma_start(xall[:, 1024:2048], x_d[:, 1024:2048])
            nc.sync.dma_start(w[:, 1536:3072], w_d[:, 1536:3072])
            nc.sync.dma_start(xall[:, 2048:4096], x_d[:, 2048:4096])
            bqk = cpool.tile([128, 2], f32, name="bqk")
            nc.sync.dma_start(bqk[:], bqk_d)
            bvo = cpool.tile([1, 512], bf16, name="bvo")
            nc.sync.dma_start(bvo[:], bvo_d)
            wot = cpool.tile([128, D], bf16, name="wot")
            nc.sync.dma_start(wot[:], wo_d)
            for t in range(1, NTC):
                nc.sync.dma_start(
                    xall[:, t * XC : (t + 1) * XC], x_d[:, t * XC : (t + 1) * XC]
                )

            ones1 = cpool.tile([1, 128], bf16, name="ones1")
            nc.vector.memset(ones1[:], 1.0)

            # ---- per-batch persistent activations ----
            qT = [cpool.tile([128, S], bf16, name=f"qT{b}") for b in range(B)]
            kT = [cpool.tile([128, S], bf16, name=f"kT{b}") for b in range(B)]
            # vaug[b]: h-major [128 keys, 2 * 16 * 65]; col 64 of each
            # 65-block is the ones column producing softmax denominators
            vaug = [cpool.tile([128, HPC * NKC * VW], bf16, name=f"va{b}") for b in range(B)]
            for b in range(B):
                nc.vector.memset(vaug[b][:, 64::VW], 1.0)

            def vslice(b, h, kc):
                return vaug[b][:, h * NKC * VW + kc * VW : h * NKC * VW + kc * VW + VW]

            for _rep in range(reps):
                # ---- phase A units (fine-grained, drained into the k-loops a
                # sub-microsecond piece at a time). q, k, v projections reuse
                # ONE [128,512] psum tile sequentially (WAR on the preceding
                # move orders them); chunk 0's q ladder streams behind the x
                # DMAs ----
                cells = {}  # t -> proj psum tile

                def unit_q_mm(t, quarter):
                    if quarter == 0:
                        cells[t] = ppool.tile(
                            [128, 512], f32, tag="proj", bufs=2, name=f"pj{t}"
                        )
                    ps = cells[t]
                    for dc in range(quarter * 2, quarter * 2 + 2):
                        nc.tensor.matmul(
                            ps[:],
                            w[:, dc * 384 : dc * 384 + 128],
                            xall[:, t * XC + dc * 512 : t * XC + (dc + 1) * 512],
                            start=(dc == 0),
                            stop=(dc == ND - 1),
                        )

                def unit_k_mm(t, quarter):
                    ps = cells[t]
                    for dc in range(quarter * 2, quarter * 2 + 2):
                        nc.tensor.matmul(
                            ps[:],
                            w[:, dc * 384 + 128 : dc * 384 + 256],
                            xall[:, t * XC + dc * 512 : t * XC + (dc + 1) * 512],
                            start=(dc == 0),
                            stop=(dc == ND - 1),
                        )

                def unit_v_mm(t, half):
                    # psum start zeroes the whole 2KB zero-region (bank), so
                    # only the FIRST quarter's first matmul starts the group;
                    # one bank-wide ones-row matmul adds the (tiled) bias and
                    # closes it
                    ps = cells[t]
                    for dc in range(half * 4, half * 4 + 4):
                        for j in range(4):
                            nc.tensor.matmul(
                                ps[:, j * 128 : (j + 1) * 128],
                                xall[:, t * XC + dc * 512 + j * 128 : t * XC + dc * 512 + (j + 1) * 128],
                                w[:, dc * 384 + 256 : dc * 384 + 384],
                                start=(dc == 0 and j == 0),
                                stop=False,
                            )
                    if half == 1:
                        nc.tensor.matmul(
                            ps[:], ones1[:], bvo[:], start=False, stop=True,
                        )

                def unit_qmove(t):
                    b, tq = t // NQC, t % NQC
                    nc.vector.tensor_scalar_add(
                        qT[b][:, tq * 512 : (tq + 1) * 512], cells[t][:], bqk[:, 0:1]
                    )

                def unit_kmove(t):
                    b, tq = t // NQC, t % NQC
                    nc.vector.tensor_scalar_add(
                        kT[b][:, tq * 512 : (tq + 1) * 512], cells[t][:], bqk[:, 1:2]
                    )

                def unit_vcopy(t):
                    b, tq = t // NQC, t % NQC
                    vp = cells.pop(t)
                    va = vaug[b]
                    dst = bass.AP(
                        va.tensor,
                        va.offset + tq * 4 * VW,
                        [va.ap[0], [NKC * VW, 2], [VW, 4], [1, 64]],
                    )
                    src = bass.AP(
                        vp.tensor, vp.offset, [vp.ap[0], [64, 2], [128, 4], [1, 64]]
                    )
                    nc.vector.tensor_copy(dst, src)

                fill = []  # (chunk, unit_idx, closure)
                N_UNITS = 13
                MOVES_DONE = 10  # units < 8: q/k matmuls + their moves

                def queue_A(t):
                    if t >= NTC:
                        return
                    units = [
                        lambda t=t: unit_q_mm(t, 0),
                        lambda t=t: unit_q_mm(t, 1),
                        lambda t=t: unit_q_mm(t, 2),
                        lambda t=t: unit_q_mm(t, 3),
                        lambda t=t: unit_qmove(t),
                        lambda t=t: unit_k_mm(t, 0),
                        lambda t=t: unit_k_mm(t, 1),
                        lambda t=t: unit_k_mm(t, 2),
                        lambda t=t: unit_k_mm(t, 3),
                        lambda t=t: unit_kmove(t),
                        lambda t=t: unit_v_mm(t, 0),
                        lambda t=t: unit_v_mm(t, 1),
                        lambda t=t: unit_vcopy(t),
                    ]
                    for u, fn in enumerate(units):
                        fill.append((t, u, fn))

                def need_A(t, n_units):
                    while fill and (
                        fill[0][0] < t or (fill[0][0] == t and fill[0][1] < n_units)
                    ):
                        fill.pop(0)[2]()

                def pop_fill():
                    if fill:
                        fill.pop(0)[2]()
                        return True
                    return False

                queued = [0]

                def ensure_queued(t):
                    while queued[0] <= min(t, NTC - 1):
                        queue_A(queued[0])
                        queued[0] += 1

                # ---- phases B/C per (batch, q-chunk) ----
                pending_first = []  # normalize muls: popped at loop start
                pending = []  # out-projection pieces: popped from i>=4

                def pop_pending():
                    if pending_first:
                        pending_first.pop(0)()
                        return True
                    if pending:
                        pending.pop(0)()
                        return True
                    return False

                def flush_pending():
                    while pending_first:
                        pending_first.pop(0)()
                    while pending:
                        pending.pop(0)()

                # qc order: the short first-chunk loop is processed LAST so
                # the long loops always have projection fill-work to absorb
                # exp-paced stretches (its A-phase is a dependency of the
                # other chunks' attention anyway)
                for b in range(B):
                    # batch 0 starts at qc=0 (its first loop then depends on
                    # one projected chunk, not two - faster start); the final
                    # batch ends on its SHORT first-chunk loop so the long
                    # loops always have projection fill-work
                    if NQC == 4 and b == B - 1:
                        qcs = [1, 2, 3, 0]
                    else:
                        qcs = list(range(NQC))
                    for qc in qcs:
                        kcs = _order_kcs(blocks, qc)
                        _lastloop = b == B - 1 and qc == qcs[-1]
                        kmax = max(kcs) // (QCH // KCH) if kcs else 0
                        req = b * NQC + max(qc, kmax)
                        ensure_queued(req)
                        need_A(req, MOVES_DONE)  # qT/kT of this chunk before scores
                        # first k-block index (emission order) whose PV reads
                        # vaug written by this chunk's own phase A
                        first_own = min(
                            (i for i, kc in enumerate(kcs) if kc // (QCH // KCH) >= qc),
                            default=len(kcs),
                        )
                        acc = ppool.tile(
                            [128, 1024], f32, tag="acc", bufs=1, name=f"acc{b}_{qc}"
                        )
                        pts = {}
                        sloppy_stop = blocks[(qc, kcs[-1])][0] == "causal" and (
                            kcs[-1] * KCH > qc * QCH
                        )

                        def emit_scores(i, b=b, qc=qc, kcs=kcs, pts=pts):
                            kc = kcs[i]
                            kind, midx = blocks[(qc, kc)]
                            f0 = 0
                            if kind == "causal":
                                f0 = max(0, kc * KCH - qc * QCH)
                            st = ppool.tile(
                                [128, 1024], f32, tag="st", bufs=2, name=f"st{b}_{qc}_{i}"
                            )
                            for h in range(HPC):
                                nc.tensor.matmul(
                                    st[:, h * 512 + f0 : (h + 1) * 512],
                                    kT[b][h * 64 : (h + 1) * 64, kc * KCH : (kc + 1) * KCH],
                                    qT[b][h * 64 : (h + 1) * 64, qc * QCH + f0 : (qc + 1) * QCH],
                                    start=True,
                                    stop=True,
                                    tile_position=(h * 64, 0),
                                )
                            pt = wpool.tile(
                                [128, 1024], bf16, tag="pt", bufs=PT_BUFS, name=f"pt{b}_{qc}_{i}"
                            )
                            if f0:
                                nc.scalar.activation(
                                    pair_ap(pt, f0, 512 - f0), pair_ap(st, f0, 512 - f0), AF.Exp
                                )
                            else:
                                nc.scalar.activation(pt[:], st[:], AF.Exp)
                            if kind == "causal":
                                nc.gpsimd.affine_select(
                                    out=pair_ap(pt, f0, 512 - f0),
                                    in_=pair_ap(pt, f0, 512 - f0),
                                    compare_op=ALU.is_ge,
                                    fill=0.0,
                                    base=qc * QCH - kc * KCH + f0,
                                    pattern=[[0, 2], [1, 512 - f0]],
                                    channel_multiplier=-1,
                                )
                            elif kind == "mixed":
                                mt = wpool.tile(
                                    [128, QCH], bf16, tag="mt", bufs=4, name=f"mt{b}_{qc}_{i}"
                                )
                                nc.sync.dma_start(mt[:], mb_d[midx * 128 : (midx + 1) * 128, :])
                                for h in range(HPC):
                                    nc.vector.tensor_mul(
                                        pt[:, h * 512 : (h + 1) * 512],
                                        pt[:, h * 512 : (h + 1) * 512],
                                        mt[:],
                                    )
                            pts[(i,)] = (pt, f0)

                        def emit_pv(i, b=b, qc=qc, kcs=kcs, pts=pts, acc=acc,
                                    sloppy=sloppy_stop, first_own=first_own, req=req):
                            if i >= first_own:
                                need_A(req, N_UNITS)  # vaug of this chunk before own-PV
                            kc = kcs[i]
                            pt, f0 = pts.pop((i,))
                            last = i == len(kcs) - 1
                            for h in range(HPC):
                                nc.tensor.matmul(
                                    acc[0:65, h * 512 + f0 : (h + 1) * 512],
                                    vslice(b, h, kc),
                                    pt[:, h * 512 + f0 : (h + 1) * 512],
                                    start=(i == 0),
                                    stop=last,
                                )

                        ensure_queued(req + 1 + LOOKAHEAD_EXTRA)
                        for i in range(len(kcs)):
                            emit_scores(i)
                            if i >= PIPE:
                                emit_pv(i - PIPE)
                            # drain next chunk's projections first (their
                            # qT/kT feed the next loop); prior chunk's
                            # deferred normalize/out-projection goes to the
                            # loop TAIL, where the score stream has ended and
                            # the psum ring + PE have slack
                            if i == 1:
                                while pending_first:
                                    pending_first.pop(0)()
                            if i >= 1:
                                got = False
                                for _ in range(FILL_POPS):
                                    got = pop_fill() or got
                                if not got and i >= PEND_AT:
                                    pop_pending()
                        for i in range(max(0, len(kcs) - PIPE), len(kcs)):
                            emit_pv(i)
                            pop_pending()

                        # ---- phase C: reciprocal + unnormalized attnT now;
                        # broadcast/normalize/out-proj deferred into the next
                        # chunk's k-loop as five sub-microsecond pieces ----
                        rec = wpool.tile([1, 1024], bf16, tag="rec", bufs=2, name=f"rc{b}{qc}")
                        attnT = wpool.tile(
                            [128, QCH], bf16, tag="attnT", bufs=2, name=f"at{b}_{qc}"
                        )
                        bcb = wpool.tile(
                            [128, 1024], bf16, tag="bcb", bufs=2, name=f"bc{b}_{qc}"
                        )
                        _last = b == B - 1 and qc == (0 if NQC == 4 else NQC - 1)
                        with nc.allow_low_precision(reason="1/denom bf16"):
                            nc.vector.reciprocal(
                                rec[:],
                                bass.AP(
                                    acc.tensor,
                                    acc.offset + 64 * acc.ap[0][0],
                                    [[acc.ap[0][0], 1], [1, 1024]],
                                ),
                            )
                        for h in range(HPC):
                            # at the very end Act is otherwise idle; elsewhere
                            # keep these off the exp stream
                            if _last:
                                nc.scalar.copy(
                                    attnT[h * 64 : (h + 1) * 64, :],
                                    acc[0:64, h * 512 : (h + 1) * 512],
                                )
                            else:
                                nc.vector.tensor_copy(
                                    attnT[h * 64 : (h + 1) * 64, :],
                                    acc[0:64, h * 512 : (h + 1) * 512],
                                )
                        nc.gpsimd.partition_broadcast(bcb[:], rec[:])
                        flush_pending()  # any leftover phase C of the prior chunk

                        osb = wpool.tile(
                            [128, 4096], bf16, tag="osb", bufs=2, name=f"ob{b}_{qc}"
                        )

                        def sub_norm(b=b, qc=qc, attnT=attnT, bcb=bcb):
                            for h in range(HPC):
                                nc.vector.tensor_mul(
                                    attnT[h * 64 : (h + 1) * 64, :],
                                    attnT[h * 64 : (h + 1) * 64, :],
                                    bcb[h * 64 : (h + 1) * 64, h * 512 : (h + 1) * 512],
                                )

                        # gpsimd cannot read PSUM, so copies out of psum are
                        # DVE/Act only; Act carries the exp stream, keep it
                        # light -- except near the end, where Act drains
                        # early and DVE is the tail's critical resource
                        _penult = b == B - 1 and qc == (NQC - 1 if NQC == 4 else max(NQC - 2, 0))
                        if _last:
                            OSB_ENG = ["act", "dve", "act", "dve"]
                        elif _penult:
                            OSB_ENG = ["act", "act", "dve", "act"]
                        else:
                            OSB_ENG = ["dve", "act", "dve", "dve"]

                        def sub_oproj(tk, b=b, qc=qc, attnT=attnT, osb=osb, _last=_last):
                            blk0 = (b * S + qc * QCH) // 128
                            op = ppool.tile(
                                [128, 1024], f32, tag="st", bufs=2, name=f"op{b}_{qc}_{tk}"
                            )
                            for oc in range(2):
                                nc.tensor.matmul(
                                    op[:, oc * 512 : (oc + 1) * 512],
                                    attnT[:, tk * 128 : (tk + 1) * 128],
                                    wot[:, oc * 512 : (oc + 1) * 512],
                                    start=True,
                                    stop=True,
                                )
                            dst = osb[:, tk * 1024 : (tk + 1) * 1024]
                            eng = OSB_ENG[tk]
                            if eng == "act":
                                nc.scalar.copy(dst, op[:])
                            elif eng == "pool":
                                nc.gpsimd.tensor_copy(dst, op[:])
                            else:
                                nc.vector.tensor_copy(dst, op[:])
                            if _last:
                                nc.sync.dma_start(
                                    out_d[:, (blk0 + tk) * 1024 : (blk0 + tk + 1) * 1024],
                                    osb[:, tk * 1024 : (tk + 1) * 1024],
                                )
                            elif tk == 3:
                                nc.sync.dma_start(
                                    out_d[:, blk0 * 1024 : (blk0 + 4) * 1024], osb[:]
                                )

                        pending_first.append(sub_norm)
                        for tk in range(4):
                            pending.append(lambda tk=tk: sub_oproj(tk))
                flush_pending()

    nc.compile()
    return nc, blocks, n_mixed


_CACHE = {}


def _get_program(mask):
    key = mask.tobytes()
    if key not in _CACHE:
        _CACHE[key] = _build(mask)
    return _CACHE[key]


def kernel(x, mask, wq, bq, wk, bk, wv, bv, wo, bo):
    x = np.asarray(x, dtype=np.float32)
    mask2 = np.asarray(mask).reshape(S, S)
    nc, blocks, n_mixed = _get_program(mask2)

    # pack x^T chunk-major: xp[p, t*4096 + dc*512 + c] = x[token t*512+c, dc*128+p]
    xp = np.ascontiguousarray(
        x.reshape(NTC, QCH, ND, 128).transpose(3, 0, 2, 1).reshape(128, NTC * XC)
    ).astype(BF)

    if n_mixed:
        mb = np.zeros((n_mixed * 128, QCH), dtype=BF)
        for (qc, kc), (kind, midx) in blocks.items():
            if kind == "mixed":
                reg = mask2[qc * QCH : (qc + 1) * QCH, kc * KCH : (kc + 1) * KCH]
                mb[midx * 128 : (midx + 1) * 128, :] = reg.T.astype(BF)

    scale = 1.0 / np.sqrt(DH)
    in_maps = []
    for c in range(NCORES):
        hsl = slice(c * HPC * DH, (c + 1) * HPC * DH)
        wq_c = np.asarray(wq)[hsl, :].T * scale  # (1024, 128)
        wk_c = np.asarray(wk)[hsl, :].T
        wv_c = np.asarray(wv)[hsl, :].T
        wqkv = np.concatenate(
            [
                np.stack([wq_c[dc * 128 : (dc + 1) * 128] for dc in range(ND)]),
                np.stack([wk_c[dc * 128 : (dc + 1) * 128] for dc in range(ND)]),
                np.stack([wv_c[dc * 128 : (dc + 1) * 128] for dc in range(ND)]),
            ],
            axis=2,
        )  # (ND, 128, 384)
        m = {
            "xp": xp,
            "wqkv": np.ascontiguousarray(
                wqkv.transpose(1, 0, 2).reshape(128, ND * 384)
            ).astype(BF),
            "bqk": np.ascontiguousarray(
                np.stack([np.asarray(bq)[hsl] * scale, np.asarray(bk)[hsl]], axis=1)
            ).astype(np.float32),
            "bvo": np.tile(np.asarray(bv)[hsl], 4).reshape(1, 512).astype(BF),
            "wot": np.ascontiguousarray(np.asarray(wo)[:, hsl].T).astype(BF),
        }
        if n_mixed:
            m["mblk"] = mb
        in_maps.append(m)

    res = run_bass_kernel_spmd(nc, in_maps, core_ids=list(range(NCORES)))
    out = np.zeros((128, (T // 128) * D), dtype=np.float64)
    for c in range(NCORES):
        out += res.results[c]["out"].astype(np.float64)
    # unpack row-block-major (128, 32*1024) -> (T, D)
    out = out.reshape(128, T // 128, D).transpose(1, 0, 2).reshape(T, D)
    out = (out + np.asarray(bo)).astype(np.float32)
    return out.reshape(B, S, D)


# revision 64
# speedup vs baseline: 1.0040x; 1.0040x over previous
"""Multi-head attention (B=2, S=2048, D=1024, H=16, causal) on 8 TRN2 cores,
head-parallel: each core computes 2 heads' q/k/v + attention and a partial
output projection; host sums the 8 partials and adds bo.

v2 (bf16): all matmul operands are bf16 (cost-model rate 1 cycle/row at any
moving size; rel-err budget 2e-2 >> bf16 error). HBM traffic is halved and
packed into a handful of large contiguous DMAs via host-side layout:

  xp   (128, 8*8*512)  x^T packed chunk-major: col = t*4096 + dc*512 + c
  wqkv (128, 8*384)    per-dc blocks [wq/sqrt(dh) | wk | wv] columns
  out  (128, 32*1024)  row-block-major: col = blk*1024 + d, blk = token//128

V is projected in flipped orientation (stationary = x sub-chunk, moving = wv
columns) so it lands directly in the [key, dim] layout attention needs - no
PE transposes. Its bias is folded in as a K=1 ones-row matmul. Scores are
computed transposed (S^T[k, q]) per head into halves of one [128,1024] psum
tile so exp / causal-select run once per k-block pair. Normalization: 1/denom
rows (from a ones-column in vaug) broadcast via a tiny ones-matmul, one fused
multiply per chunk; the k-block order puts a full-width block first (psum
start covers every column) and full-width last where possible (clean stop).
"""

import numpy as np
import ml_dtypes

import concourse.bass as bass
import concourse.tile as tile
from concourse import bacc, mybir
from concourse.bass_utils import run_bass_kernel_spmd

B, S, D, H = 2, 2048, 1024, 16
DH = D // H  # 64
NCORES = 8
HPC = H // NCORES  # 2 heads per core
T = B * S  # 4096
QCH = 512
KCH = 128
NQC = S // QCH  # 4
NKC = S // KCH  # 16
NTC = T // QCH  # 8
ND = D // 128  # 8
XC = ND * QCH  # 4096 packed-x columns per token chunk
VW = 65  # vaug block width (64 dims + ones column)

f32 = mybir.dt.float32
bf16 = mybir.dt.bfloat16
AF = mybir.ActivationFunctionType
ALU = mybir.AluOpType
BF = ml_dtypes.bfloat16

PIPE = 12  # score->exp->PV pipeline depth in k-blocks
PEND_AT = 7  # loop index from which deferred phase-C pieces are drained
FILL_POPS = 1  # phase-A units drained per k-block
LOOKAHEAD_EXTRA = 0  # extra chunks of phase-A queued beyond req+1
PT_BUFS = 14  # exp-output tiles in flight


def _classify_blocks(mask):
    """mask: (S, S) bool [q, k] -> dict (qc, kc) -> (kind, mixed_idx)."""
    blocks = {}
    qg, kg = np.meshgrid(np.arange(S), np.arange(S), indexing="ij")
    causal = qg >= kg
    n_mixed = 0
    for qc in range(NQC):
        for kc in range(NKC):
            reg = mask[qc * QCH : (qc + 1) * QCH, kc * KCH : (kc + 1) * KCH]
            if not reg.any():
                blocks[(qc, kc)] = ("none", -1)
            elif reg.all():
                blocks[(qc, kc)] = ("all", -1)
            elif np.array_equal(
                reg, causal[qc * QCH : (qc + 1) * QCH, kc * KCH : (kc + 1) * KCH]
            ):
                blocks[(qc, kc)] = ("causal", -1)
            else:
                blocks[(qc, kc)] = ("mixed", n_mixed)
                n_mixed += 1
    return blocks, n_mixed


def _order_kcs(blocks, qc):
    """k-block emission order: a full-width block first (its psum write starts
    every column), full-width blocks in the middle, and when possible a
    full-width block last (clean accumulation-group stop)."""
    kcs = [kc for kc in range(NKC) if blocks[(qc, kc)][0] != "none"]
    if not kcs:
        return []

    def f0_of(kc):
        kind, _ = blocks[(qc, kc)]
        return max(0, kc * KCH - qc * QCH) if kind == "causal" else 0

    full = [kc for kc in kcs if f0_of(kc) == 0]
    trimmed = sorted((kc for kc in kcs if f0_of(kc) > 0), key=f0_of, reverse=True)
    assert full, f"q-chunk {qc} has no full-width block"
    if len(full) == 1:
        return [full[0]] + trimmed  # sloppy stop (skip_group_check)
    return full[:-1] + trimmed + [full[-1]]


def _build(mask, reps=1):
    blocks, n_mixed = _classify_blocks(mask)

    nc = bacc.Bacc("TRN2", target_bir_lowering=False, debug=False, num_devices=NCORES)
    x_d = nc.dram_tensor("xp", (128, NTC * XC), bf16, kind="ExternalInput").ap()
    w_d = nc.dram_tensor("wqkv", (128, ND * 384), bf16, kind="ExternalInput").ap()
    bqk_d = nc.dram_tensor("bqk", (128, 2), f32, kind="ExternalInput").ap()
    bvo_d = nc.dram_tensor("bvo", (1, 512), bf16, kind="ExternalInput").ap()
    wo_d = nc.dram_tensor("wot", (128, D), bf16, kind="ExternalInput").ap()
    out_d = nc.dram_tensor("out", (128, (T // 128) * D), bf16, kind="ExternalOutput").ap()
    if n_mixed:
        mb_d = nc.dram_tensor("mblk", (n_mixed * 128, QCH), bf16, kind="ExternalInput").ap()

    def pair_ap(t, f0, width):
        """[128, (2 heads, width)] view of a [128, 1024] tile at column f0."""
        return bass.AP(t.tensor, t.offset + f0, [t.ap[0], [512, 2], [1, width]])

    with tile.TileContext(nc) as tc:
        with (
            tc.tile_pool(name="const", bufs=1) as cpool,
            tc.tile_pool(name="work", bufs=1) as wpool,
            tc.tile_pool(name="psum", bufs=1, space="PSUM") as ppool,
        ):
            # ---- input stream: interleave weights and early x so the first
            # projection matmuls start as soon as (w-dc0/1, x-chunk0) land ----
            w = cpool.tile([128, ND * 384], bf16, name="w")
            xall = cpool.tile([128, NTC * XC], bf16, name="xall")
            nc.sync.dma_start(w[:, 0:768], w_d[:, 0:768])
            nc.sync.dma_start(xall[:, 0:1024], x_d[:, 0:1024])
            nc.sync.dma_start(w[:, 768:1536], w_d[:, 768:1536])
            nc.sync.dma_start(xall[:, 1024:2048], x_d[:, 1024:2048])
            nc.sync.dma_start(w[:, 1536:3072], w_d[:, 1536:3072])
            nc.sync.dma_start(xall[:, 2048:4096], x_d[:, 2048:4096])
            bqk = cpool.tile([128, 2], f32, name="bqk")
            nc.sync.dma_start(bqk[:], bqk_d)
            bvo = cpool.tile([1, 512], bf16, name="bvo")
            nc.sync.dma_start(bvo[:], bvo_d)
            wot = cpool.tile([128, D], bf16, name="wot")
            nc.sync.dma_start(wot[:], wo_d)
            for t in range(1, NTC):
                nc.sync.dma_start(
                    xall[:, t * XC : (t + 1) * XC], x_d[:, t * XC : (t + 1) * XC]
                )

            ones1 = cpool.tile([1, 128], bf16, name="ones1")
            nc.vector.memset(ones1[:], 1.0)

            # ---- per-batch persistent activations ----
            qT = [cpool.tile([128, S], bf16, name=f"qT{b}") for b in range(B)]
            kT = [cpool.tile([128, S], bf16, name=f"kT{b}") for b in range(B)]
            # vaug[b]: h-major [128 keys, 2 * 16 * 65]; col 64 of each
            # 65-block is the ones column producing softmax denominators
            vaug = [cpool.tile([128, HPC * NKC * VW], bf16, name=f"va{b}") for b in range(B)]
            for b in range(B):
                nc.vector.memset(vaug[b][:, 64::VW], 1.0)

            def vslice(b, h, kc):
                return vaug[b][:, h * NKC * VW + kc * VW : h * NKC * VW + kc * VW + VW]

            for _rep in range(reps):
                # ---- phase A units (fine-grained, drained into the k-loops a
                # sub-microsecond piece at a time). q, k, v projections reuse
                # ONE [128,512] psum tile sequentially (WAR on the preceding
                # move orders them); chunk 0's q ladder streams behind the x
                # DMAs ----
                cells = {}  # t -> proj psum tile

                def unit_q_mm(t, quarter):
                    if quarter == 0:
                        cells[t] = ppool.tile(
                            [128, 512], f32, tag="proj", bufs=2, name=f"pj{t}"
                        )
                    ps = cells[t]
                    for dc in range(quarter * 2, quarter * 2 + 2):
                        nc.tensor.matmul(
                            ps[:],
                            w[:, dc * 384 : dc * 384 + 128],
                            xall[:, t * XC + dc * 512 : t * XC + (dc + 1) * 512],
                            start=(dc == 0),
                            stop=(dc == ND - 1),
                        )

                def unit_k_mm(t, half):
                    ps = cells[t]
                    for dc in range(half * 4, half * 4 + 4):
                        nc.tensor.matmul(
                            ps[:],
                            w[:, dc * 384 + 128 : dc * 384 + 256],
                            xall[:, t * XC + dc * 512 : t * XC + (dc + 1) * 512],
                            start=(dc == 0),
                            stop=(dc == ND - 1),
                        )

                def unit_v_mm(t, half):
                    # psum start zeroes the whole 2KB zero-region (bank), so
                    # only the FIRST quarter's first matmul starts the group;
                    # one bank-wide ones-row matmul adds the (tiled) bias and
                    # closes it
                    ps = cells[t]
                    for dc in range(half * 4, half * 4 + 4):
                        for j in range(4):
                            nc.tensor.matmul(
                                ps[:, j * 128 : (j + 1) * 128],
                                xall[:, t * XC + dc * 512 + j * 128 : t * XC + dc * 512 + (j + 1) * 128],
                                w[:, dc * 384 + 256 : dc * 384 + 384],
                                start=(dc == 0 and j == 0),
                                stop=False,
                            )
                    if half == 1:
                        nc.tensor.matmul(
                            ps[:], ones1[:], bvo[:], start=False, stop=True,
                        )

                def unit_qmove(t):
                    b, tq = t // NQC, t % NQC
                    nc.vector.tensor_scalar_add(
                        qT[b][:, tq * 512 : (tq + 1) * 512], cells[t][:], bqk[:, 0:1]
                    )

                def unit_kmove(t):
                    b, tq = t // NQC, t % NQC
                    nc.vector.tensor_scalar_add(
                        kT[b][:, tq * 512 : (tq + 1) * 512], cells[t][:], bqk[:, 1:2]
                    )

                def unit_vcopy(t):
                    b, tq = t // NQC, t % NQC
                    vp = cells.pop(t)
                    va = vaug[b]
                    dst = bass.AP(
                        va.tensor,
                        va.offset + tq * 4 * VW,
                        [va.ap[0], [NKC * VW, 2], [VW, 4], [1, 64]],
                    )
                    src = bass.AP(
                        vp.tensor, vp.offset, [vp.ap[0], [64, 2], [128, 4], [1, 64]]
                    )
                    nc.vector.tensor_copy(dst, src)

                fill = []  # (chunk, unit_idx, closure)
                N_UNITS = 11
                MOVES_DONE = 8  # units < 8: q/k matmuls + their moves

                def queue_A(t):
                    if t >= NTC:
                        return
                    units = [
                        lambda t=t: unit_q_mm(t, 0),
                        lambda t=t: unit_q_mm(t, 1),
                        lambda t=t: unit_q_mm(t, 2),
                        lambda t=t: unit_q_mm(t, 3),
                        lambda t=t: unit_qmove(t),
                        lambda t=t: unit_k_mm(t, 0),
                        lambda t=t: unit_k_mm(t, 1),
                        lambda t=t: unit_kmove(t),
                        lambda t=t: unit_v_mm(t, 0),
                        lambda t=t: unit_v_mm(t, 1),
                        lambda t=t: unit_vcopy(t),
                    ]
                    for u, fn in enumerate(units):
                        fill.append((t, u, fn))

                def need_A(t, n_units):
                    while fill and (
                        fill[0][0] < t or (fill[0][0] == t and fill[0][1] < n_units)
                    ):
                        fill.pop(0)[2]()

                def pop_fill():
                    if fill:
                        fill.pop(0)[2]()
                        return True
                    return False

                queued = [0]

                def ensure_queued(t):
                    while queued[0] <= min(t, NTC - 1):
                        queue_A(queued[0])
                        queued[0] += 1

                # ---- phases B/C per (batch, q-chunk) ----
                pending_first = []  # normalize muls: popped at loop start
                pending = []  # out-projection pieces: popped from i>=4

                def pop_pending():
                    if pending_first:
                        pending_first.pop(0)()
                        return True
                    if pending:
                        pending.pop(0)()
                        return True
                    return False

                def flush_pending():
                    while pending_first:
                        pending_first.pop(0)()
                    while pending:
                        pending.pop(0)()

                # qc order: the short first-chunk loop is processed LAST so
                # the long loops always have projection fill-work to absorb
                # exp-paced stretches (its A-phase is a dependency of the
                # other chunks' attention anyway)
                for b in range(B):
                    # batch 0 starts at qc=0 (its first loop then depends on
                    # one projected chunk, not two - faster start); the final
                    # batch ends on its SHORT first-chunk loop so the long
                    # loops always have projection fill-work
                    if NQC == 4 and b == B - 1:
                        qcs = [1, 2, 3, 0]
                    else:
                        qcs = list(range(NQC))
                    for qc in qcs:
                        kcs = _order_kcs(blocks, qc)
                        _lastloop = b == B - 1 and qc == qcs[-1]
                        kmax = max(kcs) // (QCH // KCH) if kcs else 0
                        req = b * NQC + max(qc, kmax)
                        ensure_queued(req)
                        need_A(req, MOVES_DONE)  # qT/kT of this chunk before scores
                        # first k-block index (emission order) whose PV reads
                        # vaug written by this chunk's own phase A
                        first_own = min(
                            (i for i, kc in enumerate(kcs) if kc // (QCH // KCH) >= qc),
                            default=len(kcs),
                        )
                        acc = ppool.tile(
                            [128, 1024], f32, tag="acc", bufs=1, name=f"acc{b}_{qc}"
                        )
                        pts = {}
                        sloppy_stop = blocks[(qc, kcs[-1])][0] == "causal" and (
                            kcs[-1] * KCH > qc * QCH
                        )

                        def emit_scores(i, b=b, qc=qc, kcs=kcs, pts=pts):
                            kc = kcs[i]
                            kind, midx = blocks[(qc, kc)]
                            f0 = 0
                            if kind == "causal":
                                f0 = max(0, kc * KCH - qc * QCH)
                            st = ppool.tile(
                                [128, 1024], f32, tag="st", bufs=2, name=f"st{b}_{qc}_{i}"
                            )
                            for h in range(HPC):
                                nc.tensor.matmul(
                                    st[:, h * 512 + f0 : (h + 1) * 512],
                                    kT[b][h * 64 : (h + 1) * 64, kc * KCH : (kc + 1) * KCH],
                                    qT[b][h * 64 : (h + 1) * 64, qc * QCH + f0 : (qc + 1) * QCH],
                                    start=True,
                                    stop=True,
                                    tile_position=(h * 64, 0),
                                )
                            pt = wpool.tile(
                                [128, 1024], bf16, tag="pt", bufs=PT_BUFS, name=f"pt{b}_{qc}_{i}"
                            )
                            if f0:
                                nc.scalar.activation(
                                    pair_ap(pt, f0, 512 - f0), pair_ap(st, f0, 512 - f0), AF.Exp
                                )
                            else:
                                nc.scalar.activation(pt[:], st[:], AF.Exp)
                            if kind == "causal":
                                nc.gpsimd.affine_select(
                                    out=pair_ap(pt, f0, 512 - f0),
                                    in_=pair_ap(pt, f0, 512 - f0),
                                    compare_op=ALU.is_ge,
                                    fill=0.0,
                                    base=qc * QCH - kc * KCH + f0,
                                    pattern=[[0, 2], [1, 512 - f0]],
                                    channel_multiplier=-1,
                                )
                            elif kind == "mixed":
                                mt = wpool.tile(
                                    [128, QCH], bf16, tag="mt", bufs=4, name=f"mt{b}_{qc}_{i}"
                                )
                                nc.sync.dma_start(mt[:], mb_d[midx * 128 : (midx + 1) * 128, :])
                                for h in range(HPC):
                                    nc.vector.tensor_mul(
                                        pt[:, h * 512 : (h + 1) * 512],
                                        pt[:, h * 512 : (h + 1) * 512],
                                        mt[:],
                                    )
                            pts[(i,)] = (pt, f0)

                        def emit_pv(i, b=b, qc=qc, kcs=kcs, pts=pts, acc=acc,
                                    sloppy=sloppy_stop, first_own=first_own, req=req):
                            if i >= first_own:
                                need_A(req, N_UNITS)  # vaug of this chunk before own-PV
                            kc = kcs[i]
                            pt, f0 = pts.pop((i,))
                            last = i == len(kcs) - 1
                            for h in range(HPC):
                                nc.tensor.matmul(
                                    acc[0:65, h * 512 + f0 : (h + 1) * 512],
                                    vslice(b, h, kc),
                                    pt[:, h * 512 + f0 : (h + 1) * 512],
                                    start=(i == 0),
                                    stop=last,
                                )

                        ensure_queued(req + 1 + LOOKAHEAD_EXTRA)
                        for i in range(len(kcs)):
                            emit_scores(i)
                            if i >= PIPE:
                                emit_pv(i - PIPE)
                            # drain next chunk's projections first (their
                            # qT/kT feed the next loop); prior chunk's
                            # deferred normalize/out-projection goes to the
                            # loop TAIL, where the score stream has ended and
                            # the psum ring + PE have slack
                            if i == 1:
                                while pending_first:
                                    pending_first.pop(0)()
                            if i >= 1:
                                got = False
                                for _ in range(FILL_POPS):
                                    got = pop_fill() or got
                                if not got and i >= PEND_AT:
                                    pop_pending()
                        for i in range(max(0, len(kcs) - PIPE), len(kcs)):
                            emit_pv(i)
                            pop_pending()

                        # ---- phase C: reciprocal + unnormalized attnT now;
                        # broadcast/normalize/out-proj deferred into the next
                        # chunk's k-loop as five sub-microsecond pieces ----
                        rec = wpool.tile([1, 1024], bf16, tag="rec", bufs=2, name=f"rc{b}{qc}")
                        attnT = wpool.tile(
                            [128, QCH], bf16, tag="attnT", bufs=2, name=f"at{b}_{qc}"
                        )
                        bcb = wpool.tile(
                            [128, 1024], bf16, tag="bcb", bufs=2, name=f"bc{b}_{qc}"
                        )
                        _last = b == B - 1 and qc == (0 if NQC == 4 else NQC - 1)
                        with nc.allow_low_precision(reason="1/denom bf16"):
                            nc.vector.reciprocal(
                                rec[:],
                                bass.AP(
                                    acc.tensor,
                                    acc.offset + 64 * acc.ap[0][0],
                                    [[acc.ap[0][0], 1], [1, 1024]],
                                ),
                            )
                        for h in range(HPC):
                            # at the very end Act is otherwise idle; elsewhere
                            # keep these off the exp stream
                            if _last:
                                nc.scalar.copy(
                                    attnT[h * 64 : (h + 1) * 64, :],
                                    acc[0:64, h * 512 : (h + 1) * 512],
                                )
                            else:
                                nc.vector.tensor_copy(
                                    attnT[h * 64 : (h + 1) * 64, :],
                                    acc[0:64, h * 512 : (h + 1) * 512],
                                )
                        nc.gpsimd.partition_broadcast(bcb[:], rec[:])
                        flush_pending()  # any leftover phase C of the prior chunk

                        osb = wpool.tile(
                            [128, 4096], bf16, tag="osb", bufs=2, name=f"ob{b}_{qc}"
                        )

                        def sub_norm(b=b, qc=qc, attnT=attnT, bcb=bcb):
                            for h in range(HPC):
                                nc.vector.tensor_mul(
                                    attnT[h * 64 : (h + 1) * 64, :],
                                    attnT[h * 64 : (h + 1) * 64, :],
                                    bcb[h * 64 : (h + 1) * 64, h * 512 : (h + 1) * 512],
                                )

                        # gpsimd cannot read PSUM, so copies out of psum are
                        # DVE/Act only; Act carries the exp stream, keep it
                        # light -- except near the end, where Act drains
                        # early and DVE is the tail's critical resource
                        _penult = b == B - 1 and qc == (NQC - 1 if NQC == 4 else max(NQC - 2, 0))
                        if _last:
                            OSB_ENG = ["act", "dve", "act", "dve"]
                        elif _penult:
                            OSB_ENG = ["act", "act", "dve", "act"]
                        else:
                            OSB_ENG = ["dve", "act", "dve", "dve"]

                        def sub_oproj(tk, b=b, qc=qc, attnT=attnT, osb=osb, _last=_last):
                            blk0 = (b * S + qc * QCH) // 128
                            op = ppool.tile(
                                [128, 1024], f32, tag="st", bufs=2, name=f"op{b}_{qc}_{tk}"
                            )
                            for oc in range(2):
                                nc.tensor.matmul(
                                    op[:, oc * 512 : (oc + 1) * 512],
                                    attnT[:, tk * 128 : (tk + 1) * 128],
                                    wot[:, oc * 512 : (oc + 1) * 512],
                                    start=True,
                                    stop=True,
                                )
                            dst = osb[:, tk * 1024 : (tk + 1) * 1024]
                            eng = OSB_ENG[tk]
                            if eng == "act":
                                nc.scalar.copy(dst, op[:])
                            elif eng == "pool":
                                nc.gpsimd.tensor_copy(dst, op[:])
                            else:
                                nc.vector.tensor_copy(dst, op[:])
                            if _last:
                                nc.sync.dma_start(
                                    out_d[:, (blk0 + tk) * 1024 : (blk0 + tk + 1) * 1024],
                                    osb[:, tk * 1024 : (tk + 1) * 1024],
                                )
                            elif tk == 3:
                                nc.sync.dma_start(
                                    out_d[:, blk0 * 1024 : (blk0 + 4) * 1024], osb[:]
                                )

                        pending_first.append(sub_norm)
                        for tk in range(4):
                            pending.append(lambda tk=tk: sub_oproj(tk))
                flush_pending()

    nc.compile()
    return nc, blocks, n_mixed


_CACHE = {}


def _get_program(mask):
    key = mask.tobytes()
    if key not in _CACHE:
        _CACHE[key] = _build(mask)
    return _CACHE[key]


def kernel(x, mask, wq, bq, wk, bk, wv, bv, wo, bo):
    x = np.asarray(x, dtype=np.float32)
    mask2 = np.asarray(mask).reshape(S, S)
    nc, blocks, n_mixed = _get_program(mask2)

    # pack x^T chunk-major: xp[p, t*4096 + dc*512 + c] = x[token t*512+c, dc*128+p]
    xp = np.ascontiguousarray(
        x.reshape(NTC, QCH, ND, 128).transpose(3, 0, 2, 1).reshape(128, NTC * XC)
    ).astype(BF)

    if n_mixed:
        mb = np.zeros((n_mixed * 128, QCH), dtype=BF)
        for (qc, kc), (kind, midx) in blocks.items():
            if kind == "mixed":
                reg = mask2[qc * QCH : (qc + 1) * QCH, kc * KCH : (kc + 1) * KCH]
                mb[midx * 128 : (midx + 1) * 128, :] = reg.T.astype(BF)

    scale = 1.0 / np.sqrt(DH)
    in_maps = []
    for c in range(NCORES):
        hsl = slice(c * HPC * DH, (c + 1) * HPC * DH)
        wq_c = np.asarray(wq)[hsl, :].T * scale  # (1024, 128)
        wk_c = np.asarray(wk)[hsl, :].T
        wv_c = np.asarray(wv)[hsl, :].T
        wqkv = np.concatenate(
            [
                np.stack([wq_c[dc * 128 : (dc + 1) * 128] for dc in range(ND)]),
                np.stack([wk_c[dc * 128 : (dc + 1) * 128] for dc in range(ND)]),
                np.stack([wv_c[dc * 128 : (dc + 1) * 128] for dc in range(ND)]),
            ],
            axis=2,
        )  # (ND, 128, 384)
        m = {
            "xp": xp,
            "wqkv": np.ascontiguousarray(
                wqkv.transpose(1, 0, 2).reshape(128, ND * 384)
            ).astype(BF),
            "bqk": np.ascontiguousarray(
                np.stack([np.asarray(bq)[hsl] * scale, np.asarray(bk)[hsl]], axis=1)
            ).astype(np.float32),
            "bvo": np.tile(np.asarray(bv)[hsl], 4).reshape(1, 512).astype(BF),
            "wot": np.ascontiguousarray(np.asarray(wo)[:, hsl].T).astype(BF),
        }
        if n_mixed:
            m["mblk"] = mb
        in_maps.append(m)

    res = run_bass_kernel_spmd(nc, in_maps, core_ids=list(range(NCORES)))
    out = np.zeros((128, (T // 128) * D), dtype=np.float64)
    for c in range(NCORES):
        out += res.results[c]["out"].astype(np.float64)
    # unpack row-block-major (128, 32*1024) -> (T, D)
    out = out.reshape(128, T // 128, D).transpose(1, 0, 2).reshape(T, D)
    out = (out + np.asarray(bo)).astype(np.float32)
    return out.reshape(B, S, D)


# revision 67
# speedup vs baseline: 1.0076x; 1.0036x over previous
"""Multi-head attention (B=2, S=2048, D=1024, H=16, causal) on 8 TRN2 cores,
head-parallel: each core computes 2 heads' q/k/v + attention and a partial
output projection; host sums the 8 partials and adds bo.

v2 (bf16): all matmul operands are bf16 (cost-model rate 1 cycle/row at any
moving size; rel-err budget 2e-2 >> bf16 error). HBM traffic is halved and
packed into a handful of large contiguous DMAs via host-side layout:

  xp   (128, 8*8*512)  x^T packed chunk-major: col = t*4096 + dc*512 + c
  wqkv (128, 8*384)    per-dc blocks [wq/sqrt(dh) | wk | wv] columns
  out  (128, 32*1024)  row-block-major: col = blk*1024 + d, blk = token//128

V is projected in flipped orientation (stationary = x sub-chunk, moving = wv
columns) so it lands directly in the [key, dim] layout attention needs - no
PE transposes. Its bias is folded in as a K=1 ones-row matmul. Scores are
computed transposed (S^T[k, q]) per head into halves of one [128,1024] psum
tile so exp / causal-select run once per k-block pair. Normalization: 1/denom
rows (from a ones-column in vaug) broadcast via a tiny ones-matmul, one fused
multiply per chunk; the k-block order puts a full-width block first (psum
start covers every column) and full-width last where possible (clean stop).
"""

import numpy as np
import ml_dtypes

import concourse.bass as bass
import concourse.tile as tile
from concourse import bacc, mybir
from concourse.bass_utils import run_bass_kernel_spmd

B, S, D, H = 2, 2048, 1024, 16
DH = D // H  # 64
NCORES = 8
HPC = H // NCORES  # 2 heads per core
T = B * S  # 4096
QCH = 512
KCH = 128
NQC = S // QCH  # 4
NKC = S // KCH  # 16
NTC = T // QCH  # 8
ND = D // 128  # 8
XC = ND * QCH  # 4096 packed-x columns per token chunk
VW = 65  # vaug block width (64 dims + ones column)

f32 = mybir.dt.float32
bf16 = mybir.dt.bfloat16
AF = mybir.ActivationFunctionType
ALU = mybir.AluOpType
BF = ml_dtypes.bfloat16

PIPE = 9  # score->exp->PV pipeline depth in k-blocks
PEND_AT = 6  # loop index from which deferred phase-C pieces are drained
FILL_POPS = 1  # phase-A units drained per k-block
LOOKAHEAD_EXTRA = 0  # extra chunks of phase-A queued beyond req+1
PT_BUFS = 14  # exp-output tiles in flight


def _classify_blocks(mask):
    """mask: (S, S) bool [q, k] -> dict (qc, kc) -> (kind, mixed_idx)."""
    blocks = {}
    qg, kg = np.meshgrid(np.arange(S), np.arange(S), indexing="ij")
    causal = qg >= kg
    n_mixed = 0
    for qc in range(NQC):
        for kc in range(NKC):
            reg = mask[qc * QCH : (qc + 1) * QCH, kc * KCH : (kc + 1) * KCH]
            if not reg.any():
                blocks[(qc, kc)] = ("none", -1)
            elif reg.all():
                blocks[(qc, kc)] = ("all", -1)
            elif np.array_equal(
                reg, causal[qc * QCH : (qc + 1) * QCH, kc * KCH : (kc + 1) * KCH]
            ):
                blocks[(qc, kc)] = ("causal", -1)
            else:
                blocks[(qc, kc)] = ("mixed", n_mixed)
                n_mixed += 1
    return blocks, n_mixed


def _order_kcs(blocks, qc):
    """k-block emission order: a full-width block first (its psum write starts
    every column), full-width blocks in the middle, and when possible a
    full-width block last (clean accumulation-group stop)."""
    kcs = [kc for kc in range(NKC) if blocks[(qc, kc)][0] != "none"]
    if not kcs:
        return []

    def f0_of(kc):
        kind, _ = blocks[(qc, kc)]
        return max(0, kc * KCH - qc * QCH) if kind == "causal" else 0

    full = [kc for kc in kcs if f0_of(kc) == 0]
    trimmed = sorted((kc for kc in kcs if f0_of(kc) > 0), key=f0_of, reverse=True)
    assert full, f"q-chunk {qc} has no full-width block"
    if len(full) == 1:
        return [full[0]] + trimmed  # sloppy stop (skip_group_check)
    return full[:-1] + trimmed + [full[-1]]


def _build(mask, reps=1):
    blocks, n_mixed = _classify_blocks(mask)

    nc = bacc.Bacc("TRN2", target_bir_lowering=False, debug=False, num_devices=NCORES)
    x_d = nc.dram_tensor("xp", (128, NTC * XC), bf16, kind="ExternalInput").ap()
    w_d = nc.dram_tensor("wqkv", (128, ND * 384), bf16, kind="ExternalInput").ap()
    bqk_d = nc.dram_tensor("bqk", (128, 2), f32, kind="ExternalInput").ap()
    bvo_d = nc.dram_tensor("bvo", (1, 512), bf16, kind="ExternalInput").ap()
    wo_d = nc.dram_tensor("wot", (128, D), bf16, kind="ExternalInput").ap()
    out_d = nc.dram_tensor("out", (128, (T // 128) * D), bf16, kind="ExternalOutput").ap()
    if n_mixed:
        mb_d = nc.dram_tensor("mblk", (n_mixed * 128, QCH), bf16, kind="ExternalInput").ap()

    def pair_ap(t, f0, width):
        """[128, (2 heads, width)] view of a [128, 1024] tile at column f0."""
        return bass.AP(t.tensor, t.offset + f0, [t.ap[0], [512, 2], [1, width]])

    with tile.TileContext(nc) as tc:
        with (
            tc.tile_pool(name="const", bufs=1) as cpool,
            tc.tile_pool(name="work", bufs=1) as wpool,
            tc.tile_pool(name="psum", bufs=1, space="PSUM") as ppool,
        ):
            # ---- input stream: interleave weights and early x so the first
            # projection matmuls start as soon as (w-dc0/1, x-chunk0) land ----
            w = cpool.tile([128, ND * 384], bf16, name="w")
            xall = cpool.tile([128, NTC * XC], bf16, name="xall")
            nc.sync.dma_start(w[:, 0:768], w_d[:, 0:768])
            nc.sync.dma_start(xall[:, 0:1024], x_d[:, 0:1024])
            nc.sync.dma_start(w[:, 768:1536], w_d[:, 768:1536])
            nc.sync.dma_start(xall[:, 1024:2048], x_d[:, 1024:2048])
            nc.sync.dma_start(w[:, 1536:3072], w_d[:, 1536:3072])
            nc.sync.dma_start(xall[:, 2048:4096], x_d[:, 2048:4096])
            bqk = cpool.tile([128, 2], f32, name="bqk")
            nc.sync.dma_start(bqk[:], bqk_d)
            bvo = cpool.tile([1, 512], bf16, name="bvo")
            nc.sync.dma_start(bvo[:], bvo_d)
            wot = cpool.tile([128, D], bf16, name="wot")
            nc.sync.dma_start(wot[:], wo_d)
            for t in range(1, NTC):
                nc.sync.dma_start(
                    xall[:, t * XC : (t + 1) * XC], x_d[:, t * XC : (t + 1) * XC]
                )

            ones1 = cpool.tile([1, 128], bf16, name="ones1")
            nc.vector.memset(ones1[:], 1.0)

            # warm the PE p-state during the initial DMA wait: ~3us of dummy
            # matmuls (cost-model: full clock only after 3us continuously
            # busy), so the first real projections run at full rate
            warm = ppool.tile([128, 512], f32, tag="proj", bufs=2, name="warm")
            for _ in range(10):
                nc.tensor.matmul(warm[:, 0:128], ones1[:], ones1[:], start=True, stop=True)

            # ---- per-batch persistent activations ----
            qT = [cpool.tile([128, S], bf16, name=f"qT{b}") for b in range(B)]
            kT = [cpool.tile([128, S], bf16, name=f"kT{b}") for b in range(B)]
            # vaug[b]: h-major [128 keys, 2 * 16 * 65]; col 64 of each
            # 65-block is the ones column producing softmax denominators
            vaug = [cpool.tile([128, HPC * NKC * VW], bf16, name=f"va{b}") for b in range(B)]
            for b in range(B):
                nc.vector.memset(vaug[b][:, 64::VW], 1.0)

            def vslice(b, h, kc):
                return vaug[b][:, h * NKC * VW + kc * VW : h * NKC * VW + kc * VW + VW]

            for _rep in range(reps):
                # ---- phase A units (fine-grained, drained into the k-loops a
                # sub-microsecond piece at a time). q, k, v projections reuse
                # ONE [128,512] psum tile sequentially (WAR on the preceding
                # move orders them); chunk 0's q ladder streams behind the x
                # DMAs ----
                cells = {}  # t -> proj psum tile

                def unit_q_mm(t, quarter):
                    if quarter == 0:
                        cells[t] = ppool.tile(
                            [128, 512], f32, tag="proj", bufs=2, name=f"pj{t}"
                        )
                    ps = cells[t]
                    for dc in range(quarter * 2, quarter * 2 + 2):
                        nc.tensor.matmul(
                            ps[:],
                            w[:, dc * 384 : dc * 384 + 128],
                            xall[:, t * XC + dc * 512 : t * XC + (dc + 1) * 512],
                            start=(dc == 0),
                            stop=(dc == ND - 1),
                        )

                def unit_k_mm(t, half):
                    ps = cells[t]
                    for dc in range(half * 4, half * 4 + 4):
                        nc.tensor.matmul(
                            ps[:],
                            w[:, dc * 384 + 128 : dc * 384 + 256],
                            xall[:, t * XC + dc * 512 : t * XC + (dc + 1) * 512],
                            start=(dc == 0),
                            stop=(dc == ND - 1),
                        )

                def unit_v_mm(t, half):
                    # psum start zeroes the whole 2KB zero-region (bank), so
                    # only the FIRST quarter's first matmul starts the group;
                    # one bank-wide ones-row matmul adds the (tiled) bias and
                    # closes it
                    ps = cells[t]
                    for dc in range(half * 4, half * 4 + 4):
                        for j in range(4):
                            nc.tensor.matmul(
                                ps[:, j * 128 : (j + 1) * 128],
                                xall[:, t * XC + dc * 512 + j * 128 : t * XC + dc * 512 + (j + 1) * 128],
                                w[:, dc * 384 + 256 : dc * 384 + 384],
                                start=(dc == 0 and j == 0),
                                stop=False,
                            )
                    if half == 1:
                        nc.tensor.matmul(
                            ps[:], ones1[:], bvo[:], start=False, stop=True,
                        )

                def unit_qmove(t):
                    b, tq = t // NQC, t % NQC
                    nc.vector.tensor_scalar_add(
                        qT[b][:, tq * 512 : (tq + 1) * 512], cells[t][:], bqk[:, 0:1]
                    )

                def unit_kmove(t):
                    b, tq = t // NQC, t % NQC
                    nc.vector.tensor_scalar_add(
                        kT[b][:, tq * 512 : (tq + 1) * 512], cells[t][:], bqk[:, 1:2]
                    )

                def unit_vcopy(t):
                    b, tq = t // NQC, t % NQC
                    vp = cells.pop(t)
                    va = vaug[b]
                    dst = bass.AP(
                        va.tensor,
                        va.offset + tq * 4 * VW,
                        [va.ap[0], [NKC * VW, 2], [VW, 4], [1, 64]],
                    )
                    src = bass.AP(
                        vp.tensor, vp.offset, [vp.ap[0], [64, 2], [128, 4], [1, 64]]
                    )
                    nc.vector.tensor_copy(dst, src)

                fill = []  # (chunk, unit_idx, closure)
                N_UNITS = 11
                MOVES_DONE = 8  # units < 8: q/k matmuls + their moves

                def queue_A(t):
                    if t >= NTC:
                        return
                    units = [
                        lambda t=t: unit_q_mm(t, 0),
                        lambda t=t: unit_q_mm(t, 1),
                        lambda t=t: unit_q_mm(t, 2),
                        lambda t=t: unit_q_mm(t, 3),
                        lambda t=t: unit_qmove(t),
                        lambda t=t: unit_k_mm(t, 0),
                        lambda t=t: unit_k_mm(t, 1),
                        lambda t=t: unit_kmove(t),
                        lambda t=t: unit_v_mm(t, 0),
                        lambda t=t: unit_v_mm(t, 1),
                        lambda t=t: unit_vcopy(t),
                    ]
                    for u, fn in enumerate(units):
                        fill.append((t, u, fn))

                def need_A(t, n_units):
                    while fill and (
                        fill[0][0] < t or (fill[0][0] == t and fill[0][1] < n_units)
                    ):
                        fill.pop(0)[2]()

                def pop_fill():
                    if fill:
                        fill.pop(0)[2]()
                        return True
                    return False

                queued = [0]

                def ensure_queued(t):
                    while queued[0] <= min(t, NTC - 1):
                        queue_A(queued[0])
                        queued[0] += 1

                # ---- phases B/C per (batch, q-chunk) ----
                pending_first = []  # normalize muls: popped at loop start
                pending = []  # out-projection pieces: popped from i>=4

                def pop_pending():
                    if pending_first:
                        pending_first.pop(0)()
                        return True
                    if pending:
                        pending.pop(0)()
                        return True
                    return False

                def flush_pending():
                    while pending_first:
                        pending_first.pop(0)()
                    while pending:
                        pending.pop(0)()

                # qc order: the short first-chunk loop is processed LAST so
                # the long loops always have projection fill-work to absorb
                # exp-paced stretches (its A-phase is a dependency of the
                # other chunks' attention anyway)
                for b in range(B):
                    # batch 0 starts at qc=0 (its first loop then depends on
                    # one projected chunk, not two - faster start); the final
                    # batch ends on its SHORT first-chunk loop so the long
                    # loops always have projection fill-work
                    if NQC == 4 and b == B - 1:
                        qcs = [1, 2, 3, 0]
                    else:
                        qcs = list(range(NQC))
                    for qc in qcs:
                        kcs = _order_kcs(blocks, qc)
                        _lastloop = b == B - 1 and qc == qcs[-1]
                        kmax = max(kcs) // (QCH // KCH) if kcs else 0
                        req = b * NQC + max(qc, kmax)
                        ensure_queued(req)
                        need_A(req, MOVES_DONE)  # qT/kT of this chunk before scores
                        # first k-block index (emission order) whose PV reads
                        # vaug written by this chunk's own phase A
                        first_own = min(
                            (i for i, kc in enumerate(kcs) if kc // (QCH // KCH) >= qc),
                            default=len(kcs),
                        )
                        acc = ppool.tile(
                            [128, 1024], f32, tag="acc", bufs=1, name=f"acc{b}_{qc}"
                        )
                        pts = {}
                        sloppy_stop = blocks[(qc, kcs[-1])][0] == "causal" and (
                            kcs[-1] * KCH > qc * QCH
                        )

                        def emit_scores(i, b=b, qc=qc, kcs=kcs, pts=pts):
                            kc = kcs[i]
                            kind, midx = blocks[(qc, kc)]
                            f0 = 0
                            if kind == "causal":
                                f0 = max(0, kc * KCH - qc * QCH)
                            st = ppool.tile(
                                [128, 1024], f32, tag="st", bufs=2, name=f"st{b}_{qc}_{i}"
                            )
                            for h in range(HPC):
                                nc.tensor.matmul(
                                    st[:, h * 512 + f0 : (h + 1) * 512],
                                    kT[b][h * 64 : (h + 1) * 64, kc * KCH : (kc + 1) * KCH],
                                    qT[b][h * 64 : (h + 1) * 64, qc * QCH + f0 : (qc + 1) * QCH],
                                    start=True,
                                    stop=True,
                                    tile_position=(h * 64, 0),
                                )
                            pt = wpool.tile(
                                [128, 1024], bf16, tag="pt", bufs=PT_BUFS, name=f"pt{b}_{qc}_{i}"
                            )
                            if f0:
                                nc.scalar.activation(
                                    pair_ap(pt, f0, 512 - f0), pair_ap(st, f0, 512 - f0), AF.Exp
                                )
                            else:
                                nc.scalar.activation(pt[:], st[:], AF.Exp)
                            if kind == "causal":
                                nc.gpsimd.affine_select(
                                    out=pair_ap(pt, f0, 512 - f0),
                                    in_=pair_ap(pt, f0, 512 - f0),
                                    compare_op=ALU.is_ge,
                                    fill=0.0,
                                    base=qc * QCH - kc * KCH + f0,
                                    pattern=[[0, 2], [1, 512 - f0]],
                                    channel_multiplier=-1,
                                )
                            elif kind == "mixed":
                                mt = wpool.tile(
                                    [128, QCH], bf16, tag="mt", bufs=4, name=f"mt{b}_{qc}_{i}"
                                )
                                nc.sync.dma_start(mt[:], mb_d[midx * 128 : (midx + 1) * 128, :])
                                for h in range(HPC):
                                    nc.vector.tensor_mul(
                                        pt[:, h * 512 : (h + 1) * 512],
                                        pt[:, h * 512 : (h + 1) * 512],
                                        mt[:],
                                    )
                            pts[(i,)] = (pt, f0)

                        def emit_pv(i, b=b, qc=qc, kcs=kcs, pts=pts, acc=acc,
                                    sloppy=sloppy_stop, first_own=first_own, req=req):
                            if i >= first_own:
                                need_A(req, N_UNITS)  # vaug of this chunk before own-PV
                            kc = kcs[i]
                            pt, f0 = pts.pop((i,))
                            last = i == len(kcs) - 1
                            for h in range(HPC):
                                nc.tensor.matmul(
                                    acc[0:65, h * 512 + f0 : (h + 1) * 512],
                                    vslice(b, h, kc),
                                    pt[:, h * 512 + f0 : (h + 1) * 512],
                                    start=(i == 0),
                                    stop=last,
                                )

                        ensure_queued(req + 1 + LOOKAHEAD_EXTRA)
                        for i in range(len(kcs)):
                            emit_scores(i)
                            if i >= PIPE:
                                emit_pv(i - PIPE)
                            # drain next chunk's projections first (their
                            # qT/kT feed the next loop); prior chunk's
                            # deferred normalize/out-projection goes to the
                            # loop TAIL, where the score stream has ended and
                            # the psum ring + PE have slack
                            if i == 1:
                                while pending_first:
                                    pending_first.pop(0)()
                            if i >= 1:
                                got = False
                                for _ in range(FILL_POPS):
                                    got = pop_fill() or got
                                if not got and i >= PEND_AT:
                                    pop_pending()
                        for i in range(max(0, len(kcs) - PIPE), len(kcs)):
                            emit_pv(i)
                            pop_pending()

                        # ---- phase C: reciprocal + unnormalized attnT now;
                        # broadcast/normalize/out-proj deferred into the next
                        # chunk's k-loop as five sub-microsecond pieces ----
                        rec = wpool.tile([1, 1024], bf16, tag="rec", bufs=2, name=f"rc{b}{qc}")
                        attnT = wpool.tile(
                            [128, QCH], bf16, tag="attnT", bufs=2, name=f"at{b}_{qc}"
                        )
                        bcb = wpool.tile(
                            [128, 1024], bf16, tag="bcb", bufs=2, name=f"bc{b}_{qc}"
                        )
                        _last = b == B - 1 and qc == (0 if NQC == 4 else NQC - 1)
                        with nc.allow_low_precision(reason="1/denom bf16"):
                            nc.vector.reciprocal(
                                rec[:],
                                bass.AP(
                                    acc.tensor,
                                    acc.offset + 64 * acc.ap[0][0],
                                    [[acc.ap[0][0], 1], [1, 1024]],
                                ),
                            )
                        for h in range(HPC):
                            # at the very end Act is otherwise idle; elsewhere
                            # keep these off the exp stream
                            if _last:
                                nc.scalar.copy(
                                    attnT[h * 64 : (h + 1) * 64, :],
                                    acc[0:64, h * 512 : (h + 1) * 512],
                                )
                            else:
                                nc.vector.tensor_copy(
                                    attnT[h * 64 : (h + 1) * 64, :],
                                    acc[0:64, h * 512 : (h + 1) * 512],
                                )
                        nc.gpsimd.partition_broadcast(bcb[:], rec[:])
                        flush_pending()  # any leftover phase C of the prior chunk

                        osb = wpool.tile(
                            [128, 4096], bf16, tag="osb", bufs=2, name=f"ob{b}_{qc}"
                        )

                        def sub_norm(b=b, qc=qc, attnT=attnT, bcb=bcb):
                            for h in range(HPC):
                                nc.vector.tensor_mul(
                                    attnT[h * 64 : (h + 1) * 64, :],
                                    attnT[h * 64 : (h + 1) * 64, :],
                                    bcb[h * 64 : (h + 1) * 64, h * 512 : (h + 1) * 512],
                                )

                        # gpsimd cannot read PSUM, so copies out of psum are
                        # DVE/Act only; Act carries the exp stream, keep it
                        # light -- except near the end, where Act drains
                        # early and DVE is the tail's critical resource
                        _penult = b == B - 1 and qc == (NQC - 1 if NQC == 4 else max(NQC - 2, 0))
                        if _last:
                            OSB_ENG = ["act", "dve", "act", "dve"]
                        elif _penult:
                            OSB_ENG = ["act", "act", "dve", "act"]
                        else:
                            OSB_ENG = ["dve", "act", "dve", "dve"]

                        def sub_oproj(tk, b=b, qc=qc, attnT=attnT, osb=osb, _last=_last):
                            blk0 = (b * S + qc * QCH) // 128
                            op = ppool.tile(
                                [128, 1024], f32, tag="st", bufs=2, name=f"op{b}_{qc}_{tk}"
                            )
                            for oc in range(2):
                                nc.tensor.matmul(
                                    op[:, oc * 512 : (oc + 1) * 512],
                                    attnT[:, tk * 128 : (tk + 1) * 128],
                                    wot[:, oc * 512 : (oc + 1) * 512],
                                    start=True,
                                    stop=True,
                                )
                            dst = osb[:, tk * 1024 : (tk + 1) * 1024]
                            eng = OSB_ENG[tk]
                            if eng == "act":
                                nc.scalar.copy(dst, op[:])
                            elif eng == "pool":
                                nc.gpsimd.tensor_copy(dst, op[:])
                            else:
                                nc.vector.tensor_copy(dst, op[:])
                            if _last:
                                nc.sync.dma_start(
                                    out_d[:, (blk0 + tk) * 1024 : (blk0 + tk + 1) * 1024],
                                    osb[:, tk * 1024 : (tk + 1) * 1024],
                                )
                            elif tk == 3:
                                nc.sync.dma_start(
                                    out_d[:, blk0 * 1024 : (blk0 + 4) * 1024], osb[:]
                                )

                        pending_first.append(sub_norm)
                        for tk in range(4):
                            pending.append(lambda tk=tk: sub_oproj(tk))
                flush_pending()

    nc.compile()
    return nc, blocks, n_mixed


_CACHE = {}


def _get_program(mask):
    key = mask.tobytes()
    if key not in _CACHE:
        _CACHE[key] = _build(mask)
    return _CACHE[key]


def kernel(x, mask, wq, bq, wk, bk, wv, bv, wo, bo):
    x = np.asarray(x, dtype=np.float32)
    mask2 = np.asarray(mask).reshape(S, S)
    nc, blocks, n_mixed = _get_program(mask2)

    # pack x^T chunk-major: xp[p, t*4096 + dc*512 + c] = x[token t*512+c, dc*128+p]
    xp = np.ascontiguousarray(
        x.reshape(NTC, QCH, ND, 128).transpose(3, 0, 2, 1).reshape(128, NTC * XC)
    ).astype(BF)

    if n_mixed:
        mb = np.zeros((n_mixed * 128, QCH), dtype=BF)
        for (qc, kc), (kind, midx) in blocks.items():
            if kind == "mixed":
                reg = mask2[qc * QCH : (qc + 1) * QCH, kc * KCH : (kc + 1) * KCH]
                mb[midx * 128 : (midx + 1) * 128, :] = reg.T.astype(BF)

    scale = 1.0 / np.sqrt(DH)
    in_maps = []
    for c in range(NCORES):
        hsl = slice(c * HPC * DH, (c + 1) * HPC * DH)
        wq_c = np.asarray(wq)[hsl, :].T * scale  # (1024, 128)
        wk_c = np.asarray(wk)[hsl, :].T
        wv_c = np.asarray(wv)[hsl, :].T
        wqkv = np.concatenate(
            [
                np.stack([wq_c[dc * 128 : (dc + 1) * 128] for dc in range(ND)]),
                np.stack([wk_c[dc * 128 : (dc + 1) * 128] for dc in range(ND)]),
                np.stack([wv_c[dc * 128 : (dc + 1) * 128] for dc in range(ND)]),
            ],
            axis=2,
        )  # (ND, 128, 384)
        m = {
            "xp": xp,
            "wqkv": np.ascontiguousarray(
                wqkv.transpose(1, 0, 2).reshape(128, ND * 384)
            ).astype(BF),
            "bqk": np.ascontiguousarray(
                np.stack([np.asarray(bq)[hsl] * scale, np.asarray(bk)[hsl]], axis=1)
            ).astype(np.float32),
            "bvo": np.tile(np.asarray(bv)[hsl], 4).reshape(1, 512).astype(BF),
            "wot": np.ascontiguousarray(np.asarray(wo)[:, hsl].T).astype(BF),
        }
        if n_mixed:
            m["mblk"] = mb
        in_maps.append(m)

    res = run_bass_kernel_spmd(nc, in_maps, core_ids=list(range(NCORES)))
    out = np.zeros((128, (T // 128) * D), dtype=np.float64)
    for c in range(NCORES):
        out += res.results[c]["out"].astype(np.float64)
    # unpack row-block-major (128, 32*1024) -> (T, D)
    out = out.reshape(128, T // 128, D).transpose(1, 0, 2).reshape(T, D)
    out = (out + np.asarray(bo)).astype(np.float32)
    return out.reshape(B, S, D)


# revision 83
# speedup vs baseline: 1.0213x; 1.0136x over previous
"""Multi-head attention (B=2, S=2048, D=1024, H=16, causal) on 8 TRN2 cores,
head-parallel: each core computes 2 heads' q/k/v + attention and a partial
output projection; host sums the 8 partials and adds bo.

v2 (bf16): all matmul operands are bf16 (cost-model rate 1 cycle/row at any
moving size; rel-err budget 2e-2 >> bf16 error). HBM traffic is halved and
packed into a handful of large contiguous DMAs via host-side layout:

  xp   (128, 8*8*512)  x^T packed chunk-major: col = t*4096 + dc*512 + c
  wqkv (128, 8*384)    per-dc blocks [wq/sqrt(dh) | wk | wv] columns
  out  (128, 32*1024)  row-block-major: col = blk*1024 + d, blk = token//128

V is projected in flipped orientation (stationary = x sub-chunk, moving = wv
columns) so it lands directly in the [key, dim] layout attention needs - no
PE transposes. Its bias is folded in as a K=1 ones-row matmul. Scores are
computed transposed (S^T[k, q]) per head into halves of one [128,1024] psum
tile so exp / causal-select run once per k-block pair. Normalization: 1/denom
rows (from a ones-column in vaug) are partition-broadcast on gpsimd and
multiplied into attnT on DVE; the k-block order puts a full-width block first
(psum start zeroes the whole 2KB bank - quarter-wise starts clobber each
other) and full-width last where possible (clean stop).

Scheduling: projections are split into ~0.5us units drained one per k-block
into the attention loops; deferred normalize/out-projection pieces drain from
PEND_AT; dummy warm-up matmuls during the initial DMA wait bring the PE
p-state to full clock before the first projection.
"""

import numpy as np
import ml_dtypes

import concourse.bass as bass
import concourse.tile as tile
from concourse import bacc, mybir
from concourse.bass_utils import run_bass_kernel_spmd

B, S, D, H = 2, 2048, 1024, 16
DH = D // H  # 64
NCORES = 8
HPC = H // NCORES  # 2 heads per core
T = B * S  # 4096
QCH = 512
KCH = 128
NQC = S // QCH  # 4
NKC = S // KCH  # 16
NTC = T // QCH  # 8
ND = D // 128  # 8
XC = ND * QCH  # 4096 packed-x columns per token chunk
VW = 65  # vaug block width (64 dims + ones column)

f32 = mybir.dt.float32
bf16 = mybir.dt.bfloat16
AF = mybir.ActivationFunctionType
ALU = mybir.AluOpType
BF = ml_dtypes.bfloat16

PIPE = 10  # score->exp->PV pipeline depth in k-blocks
PEND_AT = 6  # loop index from which deferred phase-C pieces are drained
FILL_POPS = 1  # phase-A units drained per k-block
LOOKAHEAD_EXTRA = 0  # extra chunks of phase-A queued beyond req+1
PT_BUFS = 14  # exp-output tiles in flight


def _classify_blocks(mask):
    """mask: (S, S) bool [q, k] -> dict (qc, kc) -> (kind, mixed_idx)."""
    blocks = {}
    qg, kg = np.meshgrid(np.arange(S), np.arange(S), indexing="ij")
    causal = qg >= kg
    n_mixed = 0
    for qc in range(NQC):
        for kc in range(NKC):
            reg = mask[qc * QCH : (qc + 1) * QCH, kc * KCH : (kc + 1) * KCH]
            if not reg.any():
                blocks[(qc, kc)] = ("none", -1)
            elif reg.all():
                blocks[(qc, kc)] = ("all", -1)
            elif np.array_equal(
                reg, causal[qc * QCH : (qc + 1) * QCH, kc * KCH : (kc + 1) * KCH]
            ):
                blocks[(qc, kc)] = ("causal", -1)
            else:
                blocks[(qc, kc)] = ("mixed", n_mixed)
                n_mixed += 1
    return blocks, n_mixed


def _order_kcs(blocks, qc):
    """k-block emission order: a full-width block first (its psum write starts
    every column), full-width blocks in the middle, and when possible a
    full-width block last (clean accumulation-group stop)."""
    kcs = [kc for kc in range(NKC) if blocks[(qc, kc)][0] != "none"]
    if not kcs:
        return []

    def f0_of(kc):
        kind, _ = blocks[(qc, kc)]
        return max(0, kc * KCH - qc * QCH) if kind == "causal" else 0

    full = [kc for kc in kcs if f0_of(kc) == 0]
    trimmed = sorted((kc for kc in kcs if f0_of(kc) > 0), key=f0_of, reverse=True)
    assert full, f"q-chunk {qc} has no full-width block"
    if len(full) == 1:
        return [full[0]] + trimmed  # sloppy stop (skip_group_check)
    return full[:-1] + trimmed + [full[-1]]


def _build(mask, reps=1):
    blocks, n_mixed = _classify_blocks(mask)

    nc = bacc.Bacc("TRN2", target_bir_lowering=False, debug=False, num_devices=NCORES)
    x_d = nc.dram_tensor("xp", (128, NTC * XC), bf16, kind="ExternalInput").ap()
    w_d = nc.dram_tensor("wqkv", (128, ND * 384), bf16, kind="ExternalInput").ap()
    bqk_d = nc.dram_tensor("bqk", (128, 2), f32, kind="ExternalInput").ap()
    wo_d = nc.dram_tensor("wot", (128, D), bf16, kind="ExternalInput").ap()
    out_d = nc.dram_tensor("out", (128, (T // 128) * D), bf16, kind="ExternalOutput").ap()
    if n_mixed:
        mb_d = nc.dram_tensor("mblk", (n_mixed * 128, QCH), bf16, kind="ExternalInput").ap()

    def pair_ap(t, f0, width):
        """[128, (2 heads, width)] view of a [128, 1024] tile at column f0."""
        return bass.AP(t.tensor, t.offset + f0, [t.ap[0], [512, 2], [1, width]])

    with tile.TileContext(nc) as tc:
        with (
            tc.tile_pool(name="const", bufs=1) as cpool,
            tc.tile_pool(name="work", bufs=1) as wpool,
            tc.tile_pool(name="psum", bufs=1, space="PSUM") as ppool,
        ):
            # ---- input stream: interleave weights and early x so the first
            # projection matmuls start as soon as (w-dc0/1, x-chunk0) land ----
            w = cpool.tile([128, ND * 384], bf16, name="w")
            xall = cpool.tile([128, NTC * XC], bf16, name="xall")
            nc.sync.dma_start(w[:, 0:768], w_d[:, 0:768])
            nc.sync.dma_start(xall[:, 0:1024], x_d[:, 0:1024])
            nc.sync.dma_start(w[:, 768:1536], w_d[:, 768:1536])
            nc.sync.dma_start(xall[:, 1024:2048], x_d[:, 1024:2048])
            nc.sync.dma_start(w[:, 1536:2304], w_d[:, 1536:2304])
            nc.sync.dma_start(xall[:, 2048:3072], x_d[:, 2048:3072])
            nc.sync.dma_start(w[:, 2304:3072], w_d[:, 2304:3072])
            nc.sync.dma_start(xall[:, 3072:4096], x_d[:, 3072:4096])
            bqk = cpool.tile([128, 2], f32, name="bqk")
            nc.sync.dma_start(bqk[:], bqk_d)
            wot = cpool.tile([128, D], bf16, name="wot")
            nc.sync.dma_start(wot[:], wo_d)
            for t in range(1, NTC):
                nc.sync.dma_start(
                    xall[:, t * XC : (t + 1) * XC], x_d[:, t * XC : (t + 1) * XC]
                )

            ones1 = cpool.tile([1, 128], bf16, name="ones1")
            nc.vector.memset(ones1[:], 1.0)

            # warm the PE p-state during the initial DMA wait: ~3us of dummy
            # matmuls (cost-model: full clock only after 3us continuously
            # busy), so the first real projections run at full rate
            warm = ppool.tile([128, 512], f32, tag="proj", bufs=2, name="warm")
            for _ in range(10):
                nc.tensor.matmul(warm[:, 0:128], ones1[:], ones1[:], start=True, stop=True)

            # ---- per-batch persistent activations ----
            qT = [cpool.tile([128, S], bf16, name=f"qT{b}") for b in range(B)]
            kT = [cpool.tile([128, S], bf16, name=f"kT{b}") for b in range(B)]
            # vaug[b]: h-major [128 keys, 2 * 16 * 65]; col 64 of each
            # 65-block is the ones column producing softmax denominators
            vaug = [cpool.tile([128, HPC * NKC * VW], bf16, name=f"va{b}") for b in range(B)]
            for b in range(B):
                nc.vector.memset(vaug[b][:, 64::VW], 1.0)

            def vslice(b, h, kc):
                return vaug[b][:, h * NKC * VW + kc * VW : h * NKC * VW + kc * VW + VW]

            for _rep in range(reps):
                # ---- phase A units (fine-grained, drained into the k-loops a
                # sub-microsecond piece at a time). q, k, v projections reuse
                # ONE [128,512] psum tile sequentially (WAR on the preceding
                # move orders them); chunk 0's q ladder streams behind the x
                # DMAs ----
                cells = {}  # t -> proj psum tile

                def unit_q_mm(t, quarter):
                    if quarter == 0:
                        cells[t] = ppool.tile(
                            [128, 512], f32, tag="proj", bufs=2, name=f"pj{t}"
                        )
                    ps = cells[t]
                    for dc in range(quarter * 2, quarter * 2 + 2):
                        nc.tensor.matmul(
                            ps[:],
                            w[:, dc * 384 : dc * 384 + 128],
                            xall[:, t * XC + dc * 512 : t * XC + (dc + 1) * 512],
                            start=(dc == 0),
                            stop=(dc == ND - 1),
                        )

                def unit_k_mm(t, half):
                    ps = cells[t]
                    for dc in range(half * 4, half * 4 + 4):
                        nc.tensor.matmul(
                            ps[:],
                            w[:, dc * 384 + 128 : dc * 384 + 256],
                            xall[:, t * XC + dc * 512 : t * XC + (dc + 1) * 512],
                            start=(dc == 0),
                            stop=(dc == ND - 1),
                        )

                def unit_v_mm(t, half):
                    # psum start zeroes the whole 2KB zero-region (bank), so
                    # only the FIRST quarter's first matmul starts the group
                    # and the last one closes it. No bias here: softmax weights
                    # sum to 1, so bv contributes the constant bv @ wo.T to the
                    # output - folded into bo on the host.
                    ps = cells[t]
                    for dc in range(half * 4, half * 4 + 4):
                        for j in range(4):
                            nc.tensor.matmul(
                                ps[:, j * 128 : (j + 1) * 128],
                                xall[:, t * XC + dc * 512 + j * 128 : t * XC + dc * 512 + (j + 1) * 128],
                                w[:, dc * 384 + 256 : dc * 384 + 384],
                                start=(dc == 0 and j == 0),
                                stop=(dc == ND - 1 and j == 3),
                            )

                def unit_qmove(t):
                    b, tq = t // NQC, t % NQC
                    nc.vector.tensor_scalar_add(
                        qT[b][:, tq * 512 : (tq + 1) * 512], cells[t][:], bqk[:, 0:1]
                    )

                def unit_kmove(t):
                    b, tq = t // NQC, t % NQC
                    nc.vector.tensor_scalar_add(
                        kT[b][:, tq * 512 : (tq + 1) * 512], cells[t][:], bqk[:, 1:2]
                    )

                def unit_vcopy(t):
                    b, tq = t // NQC, t % NQC
                    vp = cells.pop(t)
                    va = vaug[b]
                    dst = bass.AP(
                        va.tensor,
                        va.offset + tq * 4 * VW,
                        [va.ap[0], [NKC * VW, 2], [VW, 4], [1, 64]],
                    )
                    src = bass.AP(
                        vp.tensor, vp.offset, [vp.ap[0], [64, 2], [128, 4], [1, 64]]
                    )
                    nc.vector.tensor_copy(dst, src)

                fill = []  # (chunk, unit_idx, closure)
                N_UNITS = 11
                MOVES_DONE = 8  # units < 8: q/k matmuls + their moves

                def queue_A(t):
                    if t >= NTC:
                        return
                    units = [
                        lambda t=t: unit_q_mm(t, 0),
                        lambda t=t: unit_q_mm(t, 1),
                        lambda t=t: unit_q_mm(t, 2),
                        lambda t=t: unit_q_mm(t, 3),
                        lambda t=t: unit_qmove(t),
                        lambda t=t: unit_k_mm(t, 0),
                        lambda t=t: unit_k_mm(t, 1),
                        lambda t=t: unit_kmove(t),
                        lambda t=t: unit_v_mm(t, 0),
                        lambda t=t: unit_v_mm(t, 1),
                        lambda t=t: unit_vcopy(t),
                    ]
                    for u, fn in enumerate(units):
                        fill.append((t, u, fn))

                def need_A(t, n_units):
                    while fill and (
                        fill[0][0] < t or (fill[0][0] == t and fill[0][1] < n_units)
                    ):
                        fill.pop(0)[2]()

                def pop_fill():
                    if fill:
                        fill.pop(0)[2]()
                        return True
                    return False

                queued = [0]

                def ensure_queued(t):
                    while queued[0] <= min(t, NTC - 1):
                        queue_A(queued[0])
                        queued[0] += 1

                # ---- phases B/C per (batch, q-chunk) ----
                pending_first = []  # normalize muls: popped at loop start
                pending = []  # out-projection pieces: popped from i>=4

                def pop_pending():
                    if pending_first:
                        pending_first.pop(0)()
                        return True
                    if pending:
                        pending.pop(0)()
                        return True
                    return False

                def flush_pending():
                    while pending_first:
                        pending_first.pop(0)()
                    while pending:
                        pending.pop(0)()

                # qc order: the short first-chunk loop is processed LAST so
                # the long loops always have projection fill-work to absorb
                # exp-paced stretches (its A-phase is a dependency of the
                # other chunks' attention anyway)
                for b in range(B):
                    # batch 0 starts at qc=0 (its first loop then depends on
                    # one projected chunk, not two - faster start); the final
                    # batch ends on its SHORT first-chunk loop so the long
                    # loops always have projection fill-work
                    if NQC == 4 and b == B - 1:
                        qcs = [1, 2, 3, 0]
                    else:
                        qcs = list(range(NQC))
                    for qc in qcs:
                        kcs = _order_kcs(blocks, qc)
                        kmax = max(kcs) // (QCH // KCH) if kcs else 0
                        req = b * NQC + max(qc, kmax)
                        ensure_queued(req)
                        need_A(req, MOVES_DONE)  # qT/kT of this chunk before scores
                        # first k-block index (emission order) whose PV reads
                        # vaug written by this chunk's own phase A
                        first_own = min(
                            (i for i, kc in enumerate(kcs) if kc // (QCH // KCH) >= qc),
                            default=len(kcs),
                        )
                        acc = ppool.tile(
                            [128, 1024], f32, tag="acc", bufs=1, name=f"acc{b}_{qc}"
                        )
                        pts = {}
                        sloppy_stop = blocks[(qc, kcs[-1])][0] == "causal" and (
                            kcs[-1] * KCH > qc * QCH
                        )

                        def emit_scores(i, b=b, qc=qc, kcs=kcs, pts=pts):
                            kc = kcs[i]
                            kind, midx = blocks[(qc, kc)]
                            f0 = 0
                            if kind == "causal":
                                f0 = max(0, kc * KCH - qc * QCH)
                            st = ppool.tile(
                                [128, 1024], f32, tag="st", bufs=2, name=f"st{b}_{qc}_{i}"
                            )
                            for h in range(HPC):
                                nc.tensor.matmul(
                                    st[:, h * 512 + f0 : (h + 1) * 512],
                                    kT[b][h * 64 : (h + 1) * 64, kc * KCH : (kc + 1) * KCH],
                                    qT[b][h * 64 : (h + 1) * 64, qc * QCH + f0 : (qc + 1) * QCH],
                                    start=True,
                                    stop=True,
                                    tile_position=(h * 64, 0),
                                )
                            pt = wpool.tile(
                                [128, 1024], bf16, tag="pt", bufs=PT_BUFS, name=f"pt{b}_{qc}_{i}"
                            )
                            if f0:
                                nc.scalar.activation(
                                    pair_ap(pt, f0, 512 - f0), pair_ap(st, f0, 512 - f0), AF.Exp
                                )
                            else:
                                nc.scalar.activation(pt[:], st[:], AF.Exp)
                            if kind == "causal":
                                nc.gpsimd.affine_select(
                                    out=pair_ap(pt, f0, 512 - f0),
                                    in_=pair_ap(pt, f0, 512 - f0),
                                    compare_op=ALU.is_ge,
                                    fill=0.0,
                                    base=qc * QCH - kc * KCH + f0,
                                    pattern=[[0, 2], [1, 512 - f0]],
                                    channel_multiplier=-1,
                                )
                            elif kind == "mixed":
                                mt = wpool.tile(
                                    [128, QCH], bf16, tag="mt", bufs=4, name=f"mt{b}_{qc}_{i}"
                                )
                                nc.sync.dma_start(mt[:], mb_d[midx * 128 : (midx + 1) * 128, :])
                                for h in range(HPC):
                                    nc.vector.tensor_mul(
                                        pt[:, h * 512 : (h + 1) * 512],
                                        pt[:, h * 512 : (h + 1) * 512],
                                        mt[:],
                                    )
                            pts[(i,)] = (pt, f0)

                        def emit_pv(i, b=b, qc=qc, kcs=kcs, pts=pts, acc=acc,
                                    sloppy=sloppy_stop, first_own=first_own, req=req):
                            if i >= first_own:
                                need_A(req, N_UNITS)  # vaug of this chunk before own-PV
                            kc = kcs[i]
                            pt, f0 = pts.pop((i,))
                            last = i == len(kcs) - 1
                            for h in range(HPC):
                                nc.tensor.matmul(
                                    acc[0:65, h * 512 + f0 : (h + 1) * 512],
                                    vslice(b, h, kc),
                                    pt[:, h * 512 + f0 : (h + 1) * 512],
                                    start=(i == 0),
                                    stop=last,
                                )

                        ensure_queued(req + 1 + LOOKAHEAD_EXTRA)
                        for i in range(len(kcs)):
                            emit_scores(i)
                            if i >= PIPE:
                                emit_pv(i - PIPE)
                            # drain next chunk's projections first (their
                            # qT/kT feed the next loop); prior chunk's
                            # deferred normalize/out-projection goes to the
                            # loop TAIL, where the score stream has ended and
                            # the psum ring + PE have slack
                            if i == 1:
                                while pending_first:
                                    pending_first.pop(0)()
                            if i >= 1:
                                got = False
                                for _ in range(FILL_POPS):
                                    got = pop_fill() or got
                                if not got and i >= PEND_AT:
                                    pop_pending()
                        for i in range(max(0, len(kcs) - PIPE), len(kcs)):
                            emit_pv(i)
                            pop_pending()

                        # ---- phase C: reciprocal + unnormalized attnT now;
                        # broadcast/normalize/out-proj deferred into the next
                        # chunk's k-loop as five sub-microsecond pieces ----
                        rec = wpool.tile([1, 1024], bf16, tag="rec", bufs=2, name=f"rc{b}{qc}")
                        attnT = wpool.tile(
                            [128, QCH], bf16, tag="attnT", bufs=2, name=f"at{b}_{qc}"
                        )
                        bcb = wpool.tile(
                            [128, 1024], bf16, tag="bcb", bufs=2, name=f"bc{b}_{qc}"
                        )
                        _last = b == B - 1 and qc == (0 if NQC == 4 else NQC - 1)
                        with nc.allow_low_precision(reason="1/denom bf16"):
                            nc.vector.reciprocal(
                                rec[:],
                                bass.AP(
                                    acc.tensor,
                                    acc.offset + 64 * acc.ap[0][0],
                                    [[acc.ap[0][0], 1], [1, 1024]],
                                ),
                            )
                        for h in range(HPC):
                            # at the very end Act is otherwise idle; elsewhere
                            # keep these off the exp stream
                            if _last:
                                nc.scalar.copy(
                                    attnT[h * 64 : (h + 1) * 64, :],
                                    acc[0:64, h * 512 : (h + 1) * 512],
                                )
                            else:
                                nc.vector.tensor_copy(
                                    attnT[h * 64 : (h + 1) * 64, :],
                                    acc[0:64, h * 512 : (h + 1) * 512],
                                )
                        nc.gpsimd.partition_broadcast(bcb[:], rec[:])
                        flush_pending()  # any leftover phase C of the prior chunk

                        osb = wpool.tile(
                            [128, 4096], bf16, tag="osb", bufs=2, name=f"ob{b}_{qc}"
                        )

                        def sub_norm(b=b, qc=qc, attnT=attnT, bcb=bcb):
                            for h in range(HPC):
                                nc.vector.tensor_mul(
                                    attnT[h * 64 : (h + 1) * 64, :],
                                    attnT[h * 64 : (h + 1) * 64, :],
                                    bcb[h * 64 : (h + 1) * 64, h * 512 : (h + 1) * 512],
                                )

                        # gpsimd cannot read PSUM, so copies out of psum are
                        # DVE/Act only; Act carries the exp stream, keep it
                        # light -- except near the end, where Act drains
                        # early and DVE is the tail's critical resource
                        _penult = b == B - 1 and qc == (NQC - 1 if NQC == 4 else max(NQC - 2, 0))
                        if _last:
                            OSB_ENG = ["act", "dve", "act", "dve"]
                        elif _penult:
                            OSB_ENG = ["act", "act", "dve", "act"]
                        else:
                            OSB_ENG = ["dve", "act", "dve", "dve"]

                        def sub_oproj(tk, b=b, qc=qc, attnT=attnT, osb=osb, _last=_last):
                            blk0 = (b * S + qc * QCH) // 128
                            op = ppool.tile(
                                [128, 1024], f32, tag="st", bufs=2, name=f"op{b}_{qc}_{tk}"
                            )
                            for oc in range(2):
                                nc.tensor.matmul(
                                    op[:, oc * 512 : (oc + 1) * 512],
                                    attnT[:, tk * 128 : (tk + 1) * 128],
                                    wot[:, oc * 512 : (oc + 1) * 512],
                                    start=True,
                                    stop=True,
                                )
                            dst = osb[:, tk * 1024 : (tk + 1) * 1024]
                            eng = OSB_ENG[tk]
                            if eng == "act":
                                nc.scalar.copy(dst, op[:])
                            elif eng == "pool":
                                nc.gpsimd.tensor_copy(dst, op[:])
                            else:
                                nc.vector.tensor_copy(dst, op[:])
                            if _last:
                                nc.sync.dma_start(
                                    out_d[:, (blk0 + tk) * 1024 : (blk0 + tk + 1) * 1024],
                                    osb[:, tk * 1024 : (tk + 1) * 1024],
                                )
                            elif tk == 3:
                                nc.sync.dma_start(
                                    out_d[:, blk0 * 1024 : (blk0 + 4) * 1024], osb[:]
                                )

                        pending_first.append(sub_norm)
                        for tk in range(4):
                            pending.append(lambda tk=tk: sub_oproj(tk))
                flush_pending()

    nc.compile()
    return nc, blocks, n_mixed


_CACHE = {}


def _get_program(mask):
    key = mask.tobytes()
    if key not in _CACHE:
        _CACHE[key] = _build(mask)
    return _CACHE[key]


def kernel(x, mask, wq, bq, wk, bk, wv, bv, wo, bo):
    x = np.asarray(x, dtype=np.float32)
    mask2 = np.asarray(mask).reshape(S, S)
    nc, blocks, n_mixed = _get_program(mask2)

    # pack x^T chunk-major: xp[p, t*4096 + dc*512 + c] = x[token t*512+c, dc*128+p]
    xp = np.ascontiguousarray(
        x.reshape(NTC, QCH, ND, 128).transpose(3, 0, 2, 1).reshape(128, NTC * XC)
    ).astype(BF)

    if n_mixed:
        mb = np.zeros((n_mixed * 128, QCH), dtype=BF)
        for (qc, kc), (kind, midx) in blocks.items():
            if kind == "mixed":
                reg = mask2[qc * QCH : (qc + 1) * QCH, kc * KCH : (kc + 1) * KCH]
                mb[midx * 128 : (midx + 1) * 128, :] = reg.T.astype(BF)

    scale = 1.0 / np.sqrt(DH)
    in_maps = []
    for c in range(NCORES):
        hsl = slice(c * HPC * DH, (c + 1) * HPC * DH)
        wq_c = np.asarray(wq)[hsl, :].T * scale  # (1024, 128)
        wk_c = np.asarray(wk)[hsl, :].T
        wv_c = np.asarray(wv)[hsl, :].T
        wqkv = np.concatenate(
            [
                np.stack([wq_c[dc * 128 : (dc + 1) * 128] for dc in range(ND)]),
                np.stack([wk_c[dc * 128 : (dc + 1) * 128] for dc in range(ND)]),
                np.stack([wv_c[dc * 128 : (dc + 1) * 128] for dc in range(ND)]),
            ],
            axis=2,
        )  # (ND, 128, 384)
        m = {
            "xp": xp,
            "wqkv": np.ascontiguousarray(
                wqkv.transpose(1, 0, 2).reshape(128, ND * 384)
            ).astype(BF),
            "bqk": np.ascontiguousarray(
                np.stack([np.asarray(bq)[hsl] * scale, np.asarray(bk)[hsl]], axis=1)
            ).astype(np.float32),
            "wot": np.ascontiguousarray(np.asarray(wo)[:, hsl].T).astype(BF),
        }
        if n_mixed:
            m["mblk"] = mb
        in_maps.append(m)

    res = run_bass_kernel_spmd(nc, in_maps, core_ids=list(range(NCORES)))
    out = np.zeros((128, (T // 128) * D), dtype=np.float64)
    for c in range(NCORES):
        out += res.results[c]["out"].astype(np.float64)
    # unpack row-block-major (128, 32*1024) -> (T, D)
    out = out.reshape(128, T // 128, D).transpose(1, 0, 2).reshape(T, D)
    # softmax weights sum to 1, so the V bias reduces to a constant bv @ wo.T
    out = (out + np.asarray(bo) + np.asarray(bv) @ np.asarray(wo).T).astype(np.float32)
    return out.reshape(B, S, D)
